# revision 1
# baseline (speedup 1.0000x reference)
"""Trainium2 Bass kernel for nn_Net_89361089561102 (2-layer dense transformer,
NF4-quantized weights, cls head). Tensor-parallel over 8 NeuronCores.

Strategy:
 - Host: unpack NF4 weights -> bf16 partition-major [P, K/P, M]. TP sharding:
   qkv/gate_up/down/o by OUTPUT dim (heads / ff rows / H rows), so every
   projection is followed only by cheap AllGathers (reduce-collectives
   measured ~2-4x slower per byte on this fabric).
 - Full layer: per-batch token chunks pipeline through
   qkv -> attention -> ctx AG -> o (over gathered ctx) -> residual into own
   x rows -> AG of raw rows WITH the partial sum-of-squares row riding in
   the same payload (no separate ssq AllReduce on the critical path);
   consumers sum the NC gathered ssq rows, then load + rmsnorm xn in one
   in-place pass. Gated MLP: g/u -> intermediate AG -> down (over gathered
   intermediate) -> same rows+ssq AG.
 - Slim last layer: only the last token of each batch reaches the output
   through q/o/MLP; k/v full. o/gu/down evaluated at the last tokens with
   transposed matmuls (weights as the moving operand), input-sharded o/down
   plus residual folded via (partial + x_last/NC) into two tiny fp32
   AllReduces; gu/down weights in scaled fp8 (bytes, not PE rate). cls w1
   output rows are sharded across cores with a tiny AG of h; final
   rmsnorm + cls head computed redundantly per core in row form.
"""

import math
from contextlib import ExitStack
from dataclasses import dataclass

import numpy as np
import ml_dtypes

BF16 = ml_dtypes.bfloat16
FP8 = ml_dtypes.float8_e4m3
FP8_SCALE = 64.0
EPS = 1e-5
BLK = 64
NF4 = np.array([
    -1.0, -0.6961928009986877, -0.5250730514526367, -0.39491748809814453,
    -0.28444138169288635, -0.18477343022823334, -0.09105003625154495, 0.0,
    0.07958029955625534, 0.16093020141124725, 0.24611230194568634,
    0.33791524171829224, 0.44070982933044434, 0.5626170039176941,
    0.7229568362236023, 1.0], dtype=np.float32)


@dataclass(frozen=True)
class Cfg:
    H: int
    NH: int
    HD: int
    FF: int
    B: int
    S: int
    L: int
    NC: int
    CLS: int = 768
    NCLS: int = 2
    P: int = 128

    @property
    def T(self):
        return self.B * self.S

    @property
    def KT(self):
        return self.H // self.P

    @property
    def HPC(self):  # heads per core
        return self.NH // self.NC

    @property
    def DR(self):  # q/k/v rows per core (own ctx rows)
        return self.HPC * self.HD

    @property
    def DRT(self):
        return self.DR // self.P

    @property
    def OR(self):  # own x rows per core
        return self.H // self.NC

    @property
    def OT(self):
        return self.OR // self.P

    @property
    def FPC(self):  # ff rows per core
        return self.FF // self.NC

    @property
    def FT(self):
        return self.FPC // self.P

    @property
    def MT(self):  # H tiles (partial output rows)
        return self.H // self.P

    @property
    def KTF(self):  # FF contraction tiles
        return self.FF // self.P

    @property
    def SP(self):  # seq tiles per batch
        return self.S // self.P

    @property
    def TP_(self):  # token tiles total
        return self.T // self.P

    @property
    def CT(self):
        return self.CLS // self.P

    def nchunks(self, M):
        """split M free-dim into <=512 col chunks"""
        n = (M + 511) // 512
        base = M // n
        assert base * n == M
        return [(i * base, base) for i in range(n)]

    def check(self):
        assert self.H % self.P == 0 and self.FF % self.P == 0
        assert self.S % self.P == 0 and self.S <= 512
        assert self.NH % self.NC == 0 and self.H % self.NC == 0
        assert self.FF % self.NC == 0
        assert self.DR % self.P == 0, "own ctx rows must tile"
        assert self.OR % self.P == 0 and self.FPC % self.P == 0
        assert self.HD <= self.P and self.HD % 2 == 0
        assert self.CLS % self.P == 0 and self.CLS % self.NC == 0


FULL_CFG = Cfg(H=3072, NH=32, HD=96, FF=8192, B=2, S=512, L=2, NC=8)


# ----------------------------------------------------------------------------
# host-side prep
# ----------------------------------------------------------------------------

def dequant_np(packed, absmax, out_f, in_f):
    shifts = (np.arange(8, dtype=np.int32) * 4)
    codes = ((packed[:, None] >> shifts) & 0xF).reshape(-1)
    w = (NF4[codes].reshape(-1, BLK) * absmax[:, None].astype(np.float32))
    return w.reshape(out_f, in_f)


def _wpm(w_t, P, dt=BF16):
    """[K, M] fp32 -> [P, K//P, M] contiguous (partition-major)."""
    K, M = w_t.shape
    return np.ascontiguousarray(
        w_t.reshape(K // P, P, M).transpose(1, 0, 2).astype(dt))


def host_prep(cfg: Cfg, inputs):
    """Full inputs -> list of per-core input maps."""
    c = cfg
    P = c.P
    x = inputs["embed"][inputs["input_ids"]]          # [B, S, H] fp32
    x0f = np.ascontiguousarray(x.reshape(c.T, c.H).T.astype(np.float32))
    x0 = np.ascontiguousarray(x0f.astype(BF16))        # [H, T] bf16

    # rope tables
    inv = 1.0 / (10000.0 ** (np.arange(0, c.HD, 2, dtype=np.float32) / c.HD))
    f = np.outer(np.arange(c.S, dtype=np.float32), inv)
    emb = np.concatenate([f, f], -1)                   # [S, HD]
    sgn = np.concatenate([-np.ones(c.HD // 2, np.float32),
                          np.ones(c.HD // 2, np.float32)])
    cosT = np.cos(emb).T                               # [HD, S]
    sinT = np.sin(emb).T * sgn[:, None]
    cosT2 = np.ascontiguousarray(
        np.tile(cosT[:, c.S - 1:c.S], (1, c.B)).astype(np.float32))
    sinT2 = np.ascontiguousarray(
        np.tile(sinT[:, c.S - 1:c.S], (1, c.B)).astype(np.float32))
    cosT = np.ascontiguousarray(cosT.astype(np.float32))
    sinT = np.ascontiguousarray(sinT.astype(np.float32))

    am = (inputs["attention_mask"] != 0)               # [B, S]
    tk = np.arange(c.S)
    m1 = np.zeros((c.B, c.SP, P, c.S), np.float32)
    for b in range(c.B):
        for t in range(c.SP):
            rows = tk[t * P:(t + 1) * P]
            m1[b, t] = ((rows[:, None] <= tk[None, :]) & am[b, rows][:, None])
    m1 = m1.astype(BF16)
    am2 = np.zeros((c.B, P, c.SP), np.float32)
    for b in range(c.B):
        am2[b] = am[b].reshape(c.SP, P).T
    am2 = am2.astype(BF16)

    # layernorm weights, full column form [2L+1, P, KT]
    lnw = np.zeros((2 * c.L + 1, P, c.KT), np.float32)
    for l in range(c.L):
        lnw[2 * l] = inputs["ln1_w"][l].reshape(c.KT, P).T
        lnw[2 * l + 1] = inputs["ln2_w"][l].reshape(c.KT, P).T
    lnw[2 * c.L] = inputs["final_ln_w"].reshape(c.KT, P).T

    # identities for PE transposes
    id128b = np.eye(P, dtype=BF16)
    id2b = np.eye(c.B, dtype=BF16)
    id2f = np.eye(c.B, dtype=np.float32)

    # cls head (w1 output rows sharded across cores)
    CPC = c.CLS // c.NC
    gcol = np.ascontiguousarray(
        inputs["ln_g"].reshape(c.CT, P).T.astype(np.float32))
    bcol = np.ascontiguousarray(
        inputs["ln_b"].reshape(c.CT, P).T.astype(np.float32))
    w2t = _wpm(inputs["w2"].astype(np.float32).T, P)       # [P, CT, NCLS]
    b2row = np.ascontiguousarray(
        np.tile(inputs["b2"][None, :], (c.B, 1)).astype(np.float32))

    shared = dict(x0=x0, cosT=cosT, sinT=sinT, cosT2=cosT2,
                  sinT2=sinT2,
                  m1=m1, am2=am2, lnw=lnw,
                  id128b=id128b, id2b=id2b, id2f=id2f,
                  gcol=gcol, bcol=bcol, w2t=w2t,
                  b2row=b2row)

    per_layer = []
    for l in range(c.L):
        wqkv = dequant_np(inputs["qkv_packed"][l], inputs["qkv_absmax"][l],
                          3 * c.H, c.H)
        wo = dequant_np(inputs["o_packed"][l], inputs["o_absmax"][l],
                        c.H, c.H)
        wgu = dequant_np(inputs["gu_packed"][l], inputs["gu_absmax"][l],
                         2 * c.FF, c.H)
        wd = dequant_np(inputs["down_packed"][l], inputs["down_absmax"][l],
                        c.H, c.FF)
        per_layer.append((wqkv, wo, wgu, wd))

    in_maps = []
    for core in range(c.NC):
        m = dict(shared)
        m["x0r"] = np.ascontiguousarray(
            x0f[core * c.OR:(core + 1) * c.OR, :])
        m["w1t"] = _wpm(np.ascontiguousarray(
            inputs["w1"][core * CPC:(core + 1) * CPC, :].astype(
                np.float32).T), P)
        m["b1row"] = np.ascontiguousarray(np.tile(
            inputs["b1"][core * CPC:(core + 1) * CPC][None, :],
            (c.B, 1)).astype(BF16))
        lnwo = np.zeros((P, 2 * c.L + 1, c.OT), np.float32)
        for n in range(2 * c.L + 1):
            lnwo[:, n, :] = lnw[n][:, core * c.OT:(core + 1) * c.OT]
        m["lnwo"] = lnwo
        for l in range(c.L):
            wqkv, wo, wgu, wd = per_layer[l]
            d0 = core * c.DR
            m[f"wq{l}"] = _wpm(wqkv[d0:d0 + c.DR, :].T, P)
            m[f"wk{l}"] = _wpm(wqkv[c.H + d0:c.H + d0 + c.DR, :].T, P)
            m[f"wv{l}"] = _wpm(wqkv[2 * c.H + d0:2 * c.H + d0 + c.DR, :].T, P)
            o0 = core * c.OR
            g0 = core * c.FPC
            wdt = BF16 if l < c.L - 1 else FP8
            wsc = 1.0 if l < c.L - 1 else FP8_SCALE
            m[f"wg{l}"] = _wpm(wgu[g0:g0 + c.FPC, :].T * wsc, P, wdt)
            m[f"wu{l}"] = _wpm(wgu[c.FF + g0:c.FF + g0 + c.FPC, :].T * wsc,
                               P, wdt)
            if l < c.L - 1:
                m[f"wo{l}"] = _wpm(wo[o0:o0 + c.OR, :].T, P)
                m[f"wd{l}"] = _wpm(wd[o0:o0 + c.OR, :].T, P)
            else:
                m["wos"] = _wpm(
                    np.ascontiguousarray(wo[:, d0:d0 + c.DR].T), P)
                m["wds"] = _wpm(
                    np.ascontiguousarray(wd[:, g0:g0 + c.FPC].T)
                    * FP8_SCALE, P, FP8)
        in_maps.append(m)
    return in_maps


# ----------------------------------------------------------------------------
# device kernel
# ----------------------------------------------------------------------------

def build_nc(cfg: Cfg):
    import concourse.bass as bass
    import concourse.mybir as mybir
    import concourse.tile as tile
    from concourse import bacc

    c = cfg
    c.check()
    P = c.P
    f32 = mybir.dt.float32
    bf16 = mybir.dt.bfloat16
    AF = mybir.ActivationFunctionType
    OP = mybir.AluOpType

    nc = bacc.Bacc("TRN2", target_bir_lowering=False, debug=False,
                   enable_asserts=False, num_devices=c.NC)
    RG = [list(range(c.NC))]
    SHARED = "Shared" if c.NC > 4 else "Local"

    def din(name, shape, dt):
        return nc.dram_tensor(name, list(shape), dt, kind="ExternalInput").ap()

    x0 = din("x0", [c.H, c.T], bf16)
    x0r = din("x0r", [c.OR, c.T], f32)
    cosT = din("cosT", [c.HD, c.S], f32)
    sinT = din("sinT", [c.HD, c.S], f32)
    cosT2 = din("cosT2", [c.HD, c.B], f32)
    sinT2 = din("sinT2", [c.HD, c.B], f32)
    m1 = din("m1", [c.B, c.SP, P, c.S], bf16)
    am2 = din("am2", [c.B, P, c.SP], bf16)
    lnw_d = din("lnw", [2 * c.L + 1, P, c.KT], f32)
    lnwo_d = din("lnwo", [P, 2 * c.L + 1, c.OT], f32)
    id128b_d = din("id128b", [P, P], bf16)
    id2b_d = din("id2b", [c.B, c.B], bf16)
    id2f_d = din("id2f", [c.B, c.B], f32)
    CPC = c.CLS // c.NC
    w1t = din("w1t", [P, c.KT, CPC], bf16)
    b1row_d = din("b1row", [c.B, CPC], bf16)
    gcol_d = din("gcol", [P, c.CT], f32)
    bcol_d = din("bcol", [P, c.CT], f32)
    w2t_d = din("w2t", [P, c.CT, c.NCLS], bf16)
    b2row_d = din("b2row", [c.B, c.NCLS], f32)
    wq = [din(f"wq{l}", [P, c.KT, c.DR], bf16) for l in range(c.L)]
    wk = [din(f"wk{l}", [P, c.KT, c.DR], bf16) for l in range(c.L)]
    wv = [din(f"wv{l}", [P, c.KT, c.DR], bf16) for l in range(c.L)]
    wo = [din(f"wo{l}", [P, c.KT, c.OR], bf16) for l in range(c.L - 1)]
    wos = din("wos", [P, c.DRT, c.H], bf16)
    fp8 = mybir.dt.float8e4
    wg = [din(f"wg{l}", [P, c.KT, c.FPC],
              bf16 if l < c.L - 1 else fp8) for l in range(c.L)]
    wu = [din(f"wu{l}", [P, c.KT, c.FPC],
              bf16 if l < c.L - 1 else fp8) for l in range(c.L)]
    wd = [din(f"wd{l}", [P, c.KTF, c.OR], bf16) for l in range(c.L - 1)]
    wds = din("wds", [P, c.FT, c.H], fp8)
    out_d = nc.dram_tensor("logits_out", [c.B, c.NCLS], f32,
                           kind="ExternalOutput").ap()

    isqrt_hd = 1.0 / math.sqrt(c.HD)
    HCH = c.nchunks(c.H)          # H free-dim chunks (slim rows)
    FCH = c.nchunks(c.FPC)        # FPC chunks
    CCH = c.nchunks(c.CLS)        # CLS chunks

    with tile.TileContext(nc) as tc, ExitStack() as ctx:
        const = ctx.enter_context(tc.tile_pool(name="const", bufs=1))
        persist = ctx.enter_context(tc.tile_pool(name="persist", bufs=1))
        wpool = ctx.enter_context(tc.tile_pool(name="wpool", bufs=3))
        xpool = ctx.enter_context(tc.tile_pool(name="xpool", bufs=3))
        spool = ctx.enter_context(tc.tile_pool(name="spool", bufs=2))
        ppool = ctx.enter_context(tc.tile_pool(name="ppool", bufs=2))
        rpool = ctx.enter_context(tc.tile_pool(name="rpool", bufs=1))
        psum = ctx.enter_context(tc.tile_pool(name="psum", bufs=8,
                                              space="PSUM"))
        dram = ctx.enter_context(tc.tile_pool(name="dram", bufs=1,
                                              space="DRAM"))

        # ---- constants in SBUF ----
        ones_c32 = const.tile([P, 1], f32, tag="ones_c32")
        nc.vector.memset(ones_c32[:], 1.0)
        ones_cbf = const.tile([P, 1], bf16, tag="ones_cbf")
        nc.vector.memset(ones_cbf[:], 1.0)
        ones_r32 = const.tile([1, P], f32, tag="ones_r32")
        nc.vector.memset(ones_r32[:], 1.0)
        eps_col = const.tile([P, 1], f32, tag="eps_col")
        nc.vector.memset(eps_col[:], EPS)
        invnc_col = const.tile([c.B, 1], f32, tag="invnc_col")
        nc.vector.memset(invnc_col[:], 1.0 / c.NC)
        cos_sb = const.tile([c.HD, c.S], f32, tag="cos_sb")
        nc.sync.dma_start(out=cos_sb[:], in_=cosT)
        sin_sb = const.tile([c.HD, c.S], f32, tag="sin_sb")
        nc.sync.dma_start(out=sin_sb[:], in_=sinT)
        cos2_sb = const.tile([c.HD, c.B], f32, tag="cos2_sb")
        nc.sync.dma_start(out=cos2_sb[:], in_=cosT2)
        sin2_sb = const.tile([c.HD, c.B], f32, tag="sin2_sb")
        nc.sync.dma_start(out=sin2_sb[:], in_=sinT2)
        am2_sb = const.tile([P, c.B, c.SP], bf16, tag="am2_sb")
        for b in range(c.B):
            nc.sync.dma_start(out=am2_sb[:, b, :], in_=am2[b])
        lnw_sb = const.tile([P, 2 * c.L + 1, c.KT], f32, tag="lnw_sb")
        for n in range(2 * c.L + 1):
            nc.sync.dma_start(out=lnw_sb[:, n, :], in_=lnw_d[n])
        lnwo_sb = const.tile([P, 2 * c.L + 1, c.OT], f32, tag="lnwo_sb")
        nc.sync.dma_start(out=lnwo_sb[:], in_=lnwo_d)
        id128b_sb = const.tile([P, P], bf16, tag="id128b_sb")
        nc.sync.dma_start(out=id128b_sb[:], in_=id128b_d)
        id2b_sb = const.tile([c.B, c.B], bf16, tag="id2b_sb")
        nc.sync.dma_start(out=id2b_sb[:], in_=id2b_d)
        id2f_sb = const.tile([c.B, c.B], f32, tag="id2f_sb")
        nc.sync.dma_start(out=id2f_sb[:], in_=id2f_d)
        b1row_sb = const.tile([c.B, CPC], bf16, tag="b1row_sb")
        nc.sync.dma_start(out=b1row_sb[:], in_=b1row_d)
        gcol_sb = const.tile([P, c.CT], f32, tag="gcol_sb")
        nc.sync.dma_start(out=gcol_sb[:], in_=gcol_d)
        bcol_sb = const.tile([P, c.CT], f32, tag="bcol_sb")
        nc.sync.dma_start(out=bcol_sb[:], in_=bcol_d)
        w2t_sb = const.tile([P, c.CT, c.NCLS], bf16, tag="w2t_sb")
        nc.sync.dma_start(out=w2t_sb[:], in_=w2t_d)
        b2row_sb = const.tile([c.B, c.NCLS], f32, tag="b2row_sb")
        nc.sync.dma_start(out=b2row_sb[:], in_=b2row_d)

        # ---- collective warm-up: absorb channel-establish cost under
        # the first compute phase ----
        wu_sb = const.tile([P, 512], bf16, tag="wu_sb")
        nc.vector.memset(wu_sb[:], 0.0)
        wu_in = dram.tile([P, 512], bf16, tag="wu_in", name="wu_in")
        wu_out = dram.tile([P * c.NC, 512], bf16, addr_space=SHARED,
                           tag="wu_out", name="wu_out")
        nc.sync.dma_start(out=wu_in[:], in_=wu_sb[:])
        nc.gpsimd.collective_compute(
            "AllGather", OP.bypass, replica_groups=RG,
            ins=[wu_in[:]], outs=[wu_out[:]])
        wu2_in = dram.tile([c.NC, 512], bf16, tag="wu2_in", name="wu2_in")
        wu2_out = dram.tile([1, 512], bf16, tag="wu2_out", name="wu2_out")
        nc.sync.dma_start(out=wu2_in[:], in_=wu_sb[0:c.NC, :])
        nc.gpsimd.collective_compute(
            "ReduceScatter", OP.add, replica_groups=RG,
            ins=[wu2_in[:]], outs=[wu2_out[:]])
        wu3_in = dram.tile([1, 512], bf16, tag="wu3_in", name="wu3_in")
        wu3_out = dram.tile([1, 512], bf16, addr_space=SHARED,
                            tag="wu3_out", name="wu3_out")
        nc.sync.dma_start(out=wu3_in[:], in_=wu_sb[0:1, :])
        nc.gpsimd.collective_compute(
            "AllReduce", OP.add, replica_groups=RG,
            ins=[wu3_in[:]], outs=[wu3_out[:]])

        # ---- persistent activation state ----
        xn = persist.tile([P, c.KT, c.T], bf16, tag="xn")  # normalized x
        xrows = persist.tile([P, c.OT, c.T], f32, tag="xrows")  # own raw x
        for ot in range(c.OT):
            nc.sync.dma_start(out=xrows[:, ot, :],
                              in_=x0r[ot * P:(ot + 1) * P, :])

        # ---------- helpers ----------
        def emit_norm(src_ap, lnidx, ncols, col0, chunks, tag,
                      cap_dst=None, cap_col=0):
            """rmsnorm of src [H, ncols] (bf16 dram) -> xn[:, :, col0:+ncols].
            chunks: list of (c0, cw) splitting ncols for psum rows.
            cap_dst: optionally capture raw last column into [P, KT, B]."""
            ss = [psum.tile([1, cw], f32, tag="ps", name=f"ss{tag}{ci}")
                  for ci, (c0, cw) in enumerate(chunks)]
            for kt in range(c.KT):
                xf = xpool.tile([P, ncols], bf16, tag="xf", name=f"xf{tag}",
                                bufs=2)
                nc.sync.dma_start(out=xf[:], in_=src_ap[kt * P:(kt + 1) * P, :])
                nc.vector.tensor_copy(xn[:, kt, col0:col0 + ncols], xf[:])
                if cap_dst is not None:
                    nc.vector.tensor_copy(cap_dst[:, kt, cap_col:cap_col + 1],
                                          xf[:, ncols - 1:ncols])
                sq = xpool.tile([P, ncols], bf16, tag="sq", name=f"sq{tag}",
                                bufs=2)
                nc.vector.tensor_mul(sq[:], xf[:], xf[:])
                for ci, (c0, cw) in enumerate(chunks):
                    nc.tensor.matmul(ss[ci][:], ones_cbf[:], sq[:, c0:c0 + cw],
                                     start=(kt == 0), stop=(kt == c.KT - 1))
            bc = spool.tile([P, ncols], f32, tag="bc", name=f"bc{tag}",
                            bufs=1)
            for ci, (c0, cw) in enumerate(chunks):
                lt = spool.tile([1, cw], f32, tag="lt", name=f"lt{tag}",
                                bufs=1)
                nc.scalar.activation(lt[:], ss[ci][:], AF.Ln,
                                     bias=eps_col[0:1, :], scale=1.0 / c.H)
                rt = spool.tile([1, cw], f32, tag="dr", name=f"rt{tag}",
                                bufs=1)
                nc.scalar.activation(rt[:], lt[:], AF.Exp, scale=-0.5)
                bb = psum.tile([P, cw], f32, tag="ps", name=f"bb{tag}{ci}")
                nc.tensor.matmul(bb[:], ones_r32[:], rt[:],
                                 start=True, stop=True)
                nc.scalar.copy(bc[:, c0:c0 + cw], bb[:])
            for kt in range(c.KT):
                sl = xn[:, kt, col0:col0 + ncols]
                nc.vector.scalar_tensor_tensor(
                    sl, sl, lnw_sb[:, lnidx, kt:kt + 1], bc[:],
                    OP.mult, OP.mult)

        def kouter_pass(KK, wsrc, wcols, groups, rhs_fn, rhs_load=None,
                        name="kp"):
            """Contraction over KK k-tiles, streaming partition-major weights.
            groups: list of (lhs_c0, lhs_cw, out_n, rhs_key)."""
            ps = [psum.tile([cw, n], f32, tag="ps", name=f"{name}{gi}")
                  for gi, (c0, cw, n, rk) in enumerate(groups)]
            G = max(1, min(8, 2048 // wcols))
            for k0 in range(0, KK, G):
                g_n = min(G, KK - k0)
                wt = wpool.tile([P, G * wcols], bf16, tag="wt",
                                name=f"{name}w")
                wt3 = wt[:].rearrange("p (g m) -> p g m", g=G)
                nc.sync.dma_start(out=wt3[:, 0:g_n, :], in_=wsrc(k0, g_n))
                for g in range(g_n):
                    kt = k0 + g
                    rl = rhs_load(kt) if rhs_load is not None else None
                    for gi, (c0, cw, n, rk) in enumerate(groups):
                        nc.tensor.matmul(ps[gi][:], wt3[:, g, c0:c0 + cw],
                                         rhs_fn(kt, rk, rl),
                                         start=(kt == 0), stop=(kt == KK - 1))
            return ps

        def emit_rope(src_ps, qr_dst, cos_ap, sin_ap, ncols):
            """rope: qr_dst = src*cos + swap_half(src)*sin_signed."""
            h2 = c.HD // 2
            qs = rpool.tile([c.HD, ncols], bf16, tag="qs", name="qs",
                            bufs=3)
            nc.vector.tensor_copy(qs[:], src_ps[:])
            rot = rpool.tile([c.HD, ncols], bf16, tag="rot", name="rot",
                             bufs=3)
            nc.sync.dma_start(out=rot[0:h2, :], in_=qs[h2:c.HD, :])
            nc.sync.dma_start(out=rot[h2:c.HD, :], in_=qs[0:h2, :])
            nc.vector.tensor_mul(qs[:], qs[:], cos_ap)
            nc.vector.tensor_mul(rot[:], rot[:], sin_ap)
            nc.vector.tensor_add(qr_dst, qs[:], rot[:])

        def part_store(ps_list, m0, xp_dram, bcols, tag):
            """psum partial tiles (8 m-tiles starting at m0) -> DRAM rows."""
            for mi, pst in enumerate(ps_list):
                st = xpool.tile([P, bcols], bf16, tag="pst", name=f"st{tag}",
                                bufs=3)
                nc.scalar.copy(st[:], pst[:])
                r0 = (m0 + mi) * P
                nc.sync.dma_start(out=xp_dram[r0:r0 + P, :], in_=st[:])

        def own_norm_ag(b, tag):
            """AllGather this core's raw x rows PLUS its partial
            sum-of-squares row (rides in the same payload, so no separate
            ssq AllReduce sits on the critical path). Returns xg."""
            xnb = dram.tile([c.OR + 1, c.S], bf16, tag=f"xnb{tag}",
                            name=f"xnb{tag}")
            ss = psum.tile([1, c.S], f32, tag="ps", name=f"sso{tag}")
            for ot in range(c.OT):
                st = xpool.tile([P, c.S], bf16, tag="pst", name=f"sto{tag}",
                                bufs=3)
                xsl = xrows[:, ot, b * c.S:(b + 1) * c.S]
                nc.vector.tensor_copy(st[:], xsl)
                nc.sync.dma_start(out=xnb[ot * P:(ot + 1) * P, :], in_=st[:])
                sq = xpool.tile([P, c.S], bf16, tag="sq", name=f"sqo{tag}",
                                bufs=2)
                nc.vector.tensor_mul(sq[:], xsl, xsl)
                nc.tensor.matmul(ss[:], ones_cbf[:], sq[:],
                                 start=(ot == 0), stop=(ot == c.OT - 1))
            srow = spool.tile([1, c.S], bf16, tag="dr", name=f"srow{tag}",
                              bufs=1)
            nc.scalar.copy(srow[:], ss[:])
            nc.sync.dma_start(out=xnb[c.OR:c.OR + 1, :], in_=srow[:])
            xg = dram.tile([(c.OR + 1) * c.NC, c.S], bf16,
                           addr_space=SHARED, tag=f"xg{tag}",
                           name=f"xg{tag}")
            nc.gpsimd.collective_compute(
                "AllGather", OP.bypass, replica_groups=RG,
                ins=[xnb[:]], outs=[xg[:]])
            return xg

        def xn_load(xg, b, lnidx, tag):
            """Load the gathered raw x into xn chunk b, sum the NC gathered
            ssq rows, and apply the rmsnorm in one in-place pass."""
            sst = spool.tile([c.NC, c.S], bf16, tag="sst8", name=f"s8{tag}",
                             bufs=2)
            nc.sync.dma_start(
                out=sst[:],
                in_=xg[:].rearrange("(r q) s -> r q s",
                                    q=c.OR + 1)[:, c.OR, :])
            ssp = psum.tile([1, c.S], f32, tag="ps", name=f"ssp{tag}")
            nc.tensor.matmul(ssp[:], ones_cbf[0:c.NC, :], sst[:],
                             start=True, stop=True)
            lt = spool.tile([1, c.S], f32, tag="lt", name=f"lt{tag}",
                            bufs=1)
            nc.scalar.activation(lt[:], ssp[:], AF.Ln,
                                 bias=eps_col[0:1, :], scale=1.0 / c.H)
            rt = spool.tile([1, c.S], f32, tag="dr", name=f"rt{tag}",
                            bufs=1)
            nc.scalar.activation(rt[:], lt[:], AF.Exp, scale=-0.5)
            bb = psum.tile([P, c.S], f32, tag="ps", name=f"bb{tag}")
            nc.tensor.matmul(bb[:], ones_r32[:], rt[:], start=True, stop=True)
            bc = spool.tile([P, c.S], f32, tag="bc", name=f"bc{tag}",
                            bufs=1)
            nc.scalar.copy(bc[:], bb[:])
            xg3 = xg[:].rearrange("(r q) s -> r q s", q=c.OR + 1)
            for kt in range(c.KT):
                sl = xn[:, kt, b * c.S:(b + 1) * c.S]
                nc.sync.dma_start(
                    out=sl,
                    in_=xg3[kt // c.OT, (kt % c.OT) * P:(kt % c.OT + 1) * P,
                            :])
                nc.vector.scalar_tensor_tensor(
                    sl, sl, lnw_sb[:, lnidx, kt:kt + 1], bc[:],
                    OP.mult, OP.mult)

        # ================= layer 0 .. L-2 (full layers) =================
        # initial norm from replicated x0
        full_chunks = [(b * c.S, c.S) for b in range(c.B)]
        emit_norm(x0, 0, c.T, 0, full_chunks, tag="i")

        for l in range(c.L - 1):
            # ---- qkv ----
            q_rot = persist.tile([c.HD, c.HPC, c.T], bf16, tag="qrot",
                                 name=f"qrot{l}")
            k_rot = persist.tile([c.HD, c.HPC, c.T], bf16, tag="krot",
                                 name=f"krot{l}")
            v_sb = persist.tile([P, c.TP_, c.DR], bf16, tag="vsb",
                                name=f"vsb{l}")

            qg = [(h * c.HD, c.HD, c.S, b)
                  for h in range(c.HPC) for b in range(c.B)]
            qrhs = lambda kt, rk, rl: xn[:, kt, rk * c.S:(rk + 1) * c.S]
            qps = kouter_pass(c.KT, lambda k0, n: wq[l][:, k0:k0 + n, :],
                              c.DR, qg, qrhs, name="qp")
            for gi, (c0, cw, n, rk) in enumerate(qg):
                h = c0 // c.HD
                emit_rope(qps[gi], q_rot[:, h, rk * c.S:(rk + 1) * c.S],
                          cos_sb[:], sin_sb[:], c.S)
            kps = kouter_pass(c.KT, lambda k0, n: wk[l][:, k0:k0 + n, :],
                              c.DR, qg, qrhs, name="kp")
            for gi, (c0, cw, n, rk) in enumerate(qg):
                h = c0 // c.HD
                emit_rope(kps[gi], k_rot[:, h, rk * c.S:(rk + 1) * c.S],
                          cos_sb[:], sin_sb[:], c.S)
            vps = [psum.tile([P, c.DR], f32, tag="ps", name=f"vp{tt}")
                   for tt in range(c.TP_)]
            G = max(1, min(8, 2048 // c.DR))
            for k0 in range(0, c.KT, G):
                g_n = min(G, c.KT - k0)
                wt = wpool.tile([P, G * c.DR], bf16, tag="wt", name="vw")
                wt3 = wt[:].rearrange("p (g m) -> p g m", g=G)
                nc.sync.dma_start(out=wt3[:, 0:g_n, :],
                                  in_=wv[l][:, k0:k0 + g_n, :])
                for g in range(g_n):
                    kt = k0 + g
                    for tt in range(c.TP_):
                        nc.tensor.matmul(vps[tt][:],
                                         xn[:, kt, tt * P:(tt + 1) * P],
                                         wt3[:, g, :],
                                         start=(kt == 0), stop=(kt == c.KT - 1))
            for tt in range(c.TP_):
                nc.scalar.copy(v_sb[:, tt, :], vps[tt][:])

            # ---- attention (own heads -> ctxb dram, AG per chunk) ----
            ctxgs = []
            for b in range(c.B):
                ctxb = dram.tile([c.DR, c.S], bf16, tag=f"ctxb{l}_{b}",
                                 name=f"ctxb{l}_{b}")
                mask_sb = ppool.tile([P, c.SP, c.S], bf16, tag="maskb",
                                     name=f"maskb{l}{b}", bufs=2)
                for t in range(c.SP):
                    nc.sync.dma_start(out=mask_sb[:, t, :], in_=m1[b, t])
                for h in range(c.HPC):
                    den = psum.tile([1, c.S], f32, tag="ps", name="den")
                    cps = psum.tile([c.HD, c.S], f32, tag="ps", name="cps")
                    for t in range(c.SP):
                        sps = psum.tile([P, c.S], f32, tag="ps", name="sps")
                        nc.tensor.matmul(
                            sps[:],
                            k_rot[:, h, b * c.S + t * P:
                                  b * c.S + (t + 1) * P],
                            q_rot[:, h, b * c.S:(b + 1) * c.S],
                            start=True, stop=True)
                        pt = ppool.tile([P, c.S], bf16, tag="pt", name="pt",
                                        bufs=3)
                        nc.scalar.activation(pt[:], sps[:], AF.Exp,
                                             scale=isqrt_hd)
                        nc.vector.tensor_mul(
                            pt[:], pt[:], mask_sb[:, t, :])
                        nc.tensor.matmul(den[:], ones_cbf[:], pt[:],
                                         start=(t == 0),
                                         stop=(t == c.SP - 1))
                        nc.tensor.matmul(
                            cps[:],
                            v_sb[:, b * c.SP + t,
                                 h * c.HD:(h + 1) * c.HD],
                            pt[:],
                            start=(t == 0), stop=(t == c.SP - 1))
                    dr = spool.tile([1, c.S], f32, tag="dr", name="dr",
                                    bufs=1)
                    nc.vector.reciprocal(dr[:], den[:])
                    bb = psum.tile([c.HD, c.S], f32, tag="ps", name="bb")
                    nc.tensor.matmul(bb[:], ones_r32[:, 0:c.HD], dr[:],
                                     start=True, stop=True)
                    bsb = spool.tile([c.HD, c.S], bf16, tag="bsb",
                                     name="bsb", bufs=2)
                    nc.vector.tensor_copy(bsb[:], bb[:])
                    csb = spool.tile([c.HD, c.S], bf16, tag="csb",
                                     name="csb", bufs=1)
                    nc.vector.tensor_mul(csb[:], cps[:], bsb[:])
                    nc.sync.dma_start(
                        out=ctxb[h * c.HD:(h + 1) * c.HD, :], in_=csb[:])
                ctxg = dram.tile([c.H, c.S], bf16, addr_space=SHARED,
                                 tag=f"ctxg{l}_{b}", name=f"ctxg{l}_{b}")
                nc.gpsimd.collective_compute(
                    "AllGather", OP.bypass, replica_groups=RG,
                    ins=[ctxb[:]], outs=[ctxg[:]])
                ctxgs.append(ctxg)

            # ---- o (output-sharded over gathered ctx) + residual + norm ----
            xgs = []
            for b in range(c.B):
                og = [(ot * P, P, c.S, b) for ot in range(c.OT)]

                def oload(kt, _b=b):
                    t = xpool.tile([P, c.S], bf16, tag="rhs", name="orhs",
                                   bufs=5)
                    nc.scalar.dma_start(
                        out=t[:], in_=ctxgs[_b][kt * P:(kt + 1) * P, :])
                    return t
                ops_ = kouter_pass(c.KT, lambda k0, n: wo[l][:, k0:k0 + n, :],
                                   c.OR, og, lambda kt, rk, rl: rl[:],
                                   rhs_load=oload, name=f"op{b}")
                for ot in range(c.OT):
                    xsl = xrows[:, ot, b * c.S:(b + 1) * c.S]
                    nc.vector.tensor_add(xsl, xsl, ops_[ot][:])
                xgs.append(own_norm_ag(b, tag=f"o{l}_{b}"))

            # ---- gated MLP per chunk: xn load -> g/u -> int AG -> down ----
            gact = persist.tile([P, c.FT, c.S], bf16, tag="gact",
                                name=f"gact{l}")
            intgs = []
            for b in range(c.B):
                xn_load(xgs[b], b, 2 * l + 1, tag=f"xo{l}{b}")
                intb = dram.tile([c.FPC, c.S], bf16, tag=f"intb{l}_{b}",
                                 name=f"intb{l}_{b}")
                for phase, wsrc3 in (("g", wg[l]), ("u", wu[l])):
                    gg = [(ot * P, P, c.S, b) for ot in range(c.FT)]
                    grhs = (lambda kt, rk, rl:
                            xn[:, kt, rk * c.S:(rk + 1) * c.S])
                    gps = kouter_pass(
                        c.KT, lambda k0, n, _w=wsrc3: _w[:, k0:k0 + n, :],
                        c.FPC, gg, grhs, name=f"{phase}{l}{b}")
                    for gi, (c0, cw, n, rk) in enumerate(gg):
                        ot = c0 // P
                        if phase == "g":
                            sgt = xpool.tile([P, c.S], bf16, tag="sgt",
                                             name="sgt", bufs=3)
                            nc.scalar.activation(sgt[:], gps[gi][:],
                                                 AF.Sigmoid)
                            nc.vector.tensor_mul(gact[:, ot, :], gps[gi][:],
                                                 sgt[:])
                        else:
                            it = xpool.tile([P, c.S], bf16, tag="pst",
                                            name="it", bufs=3)
                            nc.vector.tensor_mul(it[:], gps[gi][:],
                                                 gact[:, ot, :])
                            nc.sync.dma_start(
                                out=intb[ot * P:(ot + 1) * P, :], in_=it[:])
                intg = dram.tile([c.FF, c.S], bf16, addr_space=SHARED,
                                 tag=f"intg{l}_{b}", name=f"intg{l}_{b}")
                nc.gpsimd.collective_compute(
                    "AllGather", OP.bypass, replica_groups=RG,
                    ins=[intb[:]], outs=[intg[:]])
                intgs.append(intg)

            # ---- down (output-sharded over gathered intermediate) ----
            xgds = []
            for b in range(c.B):
                dg = [(ot * P, P, c.S, b) for ot in range(c.OT)]

                def dload(kt, _b=b):
                    t = xpool.tile([P, c.S], bf16, tag="rhs", name="drhs",
                                   bufs=5)
                    nc.scalar.dma_start(
                        out=t[:], in_=intgs[_b][kt * P:(kt + 1) * P, :])
                    return t
                dps_ = kouter_pass(c.KTF,
                                   lambda k0, n: wd[l][:, k0:k0 + n, :],
                                   c.OR, dg, lambda kt, rk, rl: rl[:],
                                   rhs_load=dload, name=f"dp{b}")
                for ot in range(c.OT):
                    xsl = xrows[:, ot, b * c.S:(b + 1) * c.S]
                    nc.vector.tensor_add(xsl, xsl, dps_[ot][:])
                xgds.append(own_norm_ag(b, tag=f"d{l}_{b}"))

        # ================= slim last layer =================
        l = c.L - 1

        # raw x at the last token of each batch (for the slim residual):
        # tiny AG of own rows' last columns
        xlb = dram.tile([c.OR, c.B], bf16, tag="xlb", name="xlb")
        for ot in range(c.OT):
            st = xpool.tile([P, c.B], bf16, tag="xlst", name="xlst", bufs=2)
            nc.vector.tensor_copy(
                st[:],
                xrows[:, ot, :].rearrange("p (b s) -> p b s",
                                          s=c.S)[:, :, c.S - 1])
            nc.sync.dma_start(out=xlb[ot * P:(ot + 1) * P, :], in_=st[:])
        xlg_raw = dram.tile([c.H, c.B], bf16, addr_space=SHARED,
                            tag="xlg_raw", name="xlg_raw")
        nc.gpsimd.collective_compute(
            "AllGather", OP.bypass, replica_groups=RG,
            ins=[xlb[:]], outs=[xlg_raw[:]])
        xlraw = persist.tile([P, c.KT, c.B], bf16, tag="xlraw",
                             name="xlraw")
        nc.sync.dma_start(
            out=xlraw[:],
            in_=xlg_raw[:].rearrange("(kt p) b -> p kt b", p=P))

        # ---- down-transition xn load chunk b + k/v pass chunk b ----
        q_rot2 = persist.tile([c.HD, c.HPC, c.B], bf16, tag="qrot2",
                              name="qrot2")
        k_rot = persist.tile([c.HD, c.HPC, c.T], bf16, tag="krot",
                             name=f"krot{l}")
        v_sb = persist.tile([P, c.TP_, c.DR], bf16, tag="vsb",
                            name=f"vsb{l}")
        for b in range(c.B):
            xn_load(xgds[b], b, 2 * l, tag=f"xd{b}")
            kg = [(h * c.HD, c.HD, c.S, b) for h in range(c.HPC)]
            krhs = lambda kt, rk, rl: xn[:, kt, rk * c.S:(rk + 1) * c.S]
            kps = kouter_pass(c.KT, lambda k0, n: wk[l][:, k0:k0 + n, :],
                              c.DR, kg, krhs, name=f"kp2{b}")
            for gi, (c0, cw, n, rk) in enumerate(kg):
                h = c0 // c.HD
                emit_rope(kps[gi], k_rot[:, h, rk * c.S:(rk + 1) * c.S],
                          cos_sb[:], sin_sb[:], c.S)
            vps = [psum.tile([P, c.DR], f32, tag="ps", name=f"vp2{b}{tt}")
                   for tt in range(c.SP)]
            G = max(1, min(8, 2048 // c.DR))
            for k0 in range(0, c.KT, G):
                g_n = min(G, c.KT - k0)
                wt = wpool.tile([P, G * c.DR], bf16, tag="wt", name="vw2")
                wt3 = wt[:].rearrange("p (g m) -> p g m", g=G)
                nc.sync.dma_start(out=wt3[:, 0:g_n, :],
                                  in_=wv[l][:, k0:k0 + g_n, :])
                for g in range(g_n):
                    kt = k0 + g
                    for tt in range(c.SP):
                        tg = b * c.SP + tt
                        nc.tensor.matmul(vps[tt][:],
                                         xn[:, kt, tg * P:(tg + 1) * P],
                                         wt3[:, g, :],
                                         start=(kt == 0), stop=(kt == c.KT - 1))
            for tt in range(c.SP):
                nc.scalar.copy(v_sb[:, b * c.SP + tt, :], vps[tt][:])

        # slim q: transposed pass -> qT [B, DR], then per-head transpose+rope
        xnl = persist.tile([P, c.KT, c.B], bf16, tag="xnl", name="xnl")
        nc.vector.tensor_copy(
            xnl[:],
            xn[:].rearrange("p kt (b s) -> p kt b s", s=c.S)[:, :, :, c.S - 1])
        qTp = psum.tile([c.B, c.DR], f32, tag="ps", name="qTp")
        G = max(1, min(8, 2048 // c.DR))
        for k0 in range(0, c.KT, G):
            g_n = min(G, c.KT - k0)
            wt = wpool.tile([P, G * c.DR], bf16, tag="wt", name="qw2")
            wt3 = wt[:].rearrange("p (g m) -> p g m", g=G)
            nc.sync.dma_start(out=wt3[:, 0:g_n, :],
                              in_=wq[l][:, k0:k0 + g_n, :])
            for g in range(g_n):
                kt = k0 + g
                nc.tensor.matmul(qTp[:], xnl[:, kt, :], wt3[:, g, :],
                                 start=(kt == 0), stop=(kt == c.KT - 1))
        qTr = spool.tile([c.B, c.DR], f32, tag="qTr", name="qTr", bufs=1)
        nc.vector.tensor_copy(qTr[:], qTp[:])
        for h in range(c.HPC):
            qhp = psum.tile([c.HD, c.B], f32, tag="ps", name=f"qhp{h}")
            nc.tensor.matmul(qhp[:], qTr[:, h * c.HD:(h + 1) * c.HD],
                             id2f_sb[:], is_transpose=True,
                             start=True, stop=True)
            emit_rope(qhp, q_rot2[:, h, :], cos2_sb[:], sin2_sb[:], c.B)

        # ---- slim attention -> ctx_lastT tiles [P, DRT, B] ----
        ctxL = persist.tile([P, c.DRT, c.B], bf16, tag="ctxL", name="ctxL")
        for b in range(c.B):
            for h in range(c.HPC):
                sps = psum.tile([P, c.SP], f32, tag="ps", name="sps2")
                for t in range(c.SP):
                    nc.tensor.matmul(
                        sps[:, t:t + 1],
                        k_rot[:, h, b * c.S + t * P:b * c.S + (t + 1) * P],
                        q_rot2[:, h, b:b + 1],
                        start=True, stop=True)
                pt = ppool.tile([P, c.SP], bf16, tag="pt", name="pt2",
                                bufs=3)
                nc.scalar.activation(pt[:], sps[:], AF.Exp, scale=isqrt_hd)
                nc.vector.tensor_mul(pt[:], pt[:], am2_sb[:, b, :])
                dps = psum.tile([1, c.SP], f32, tag="ps", name="dps")
                nc.tensor.matmul(dps[:], ones_cbf[:], pt[:],
                                 start=True, stop=True)
                d1 = spool.tile([1, 1], f32, tag="d1", name="d1")
                nc.vector.tensor_reduce(d1[:], dps[:],
                                        mybir.AxisListType.X, OP.add)
                r1 = spool.tile([1, 1], f32, tag="r1", name="r1")
                nc.vector.reciprocal(r1[:], d1[:])
                cps = psum.tile([c.HD, 1], f32, tag="ps", name="cps2")
                for t in range(c.SP):
                    nc.tensor.matmul(
                        cps[:],
                        v_sb[:, b * c.SP + t, h * c.HD:(h + 1) * c.HD],
                        pt[:, t:t + 1],
                        start=(t == 0), stop=(t == c.SP - 1))
                bb = psum.tile([c.HD, 1], f32, tag="ps", name="bb2")
                nc.tensor.matmul(bb[:], ones_r32[:, 0:c.HD], r1[:],
                                 start=True, stop=True)
                bsb = spool.tile([c.HD, 1], f32, tag="bsb2", name="bsb2")
                nc.vector.tensor_copy(bsb[:], bb[:])
                csb = spool.tile([c.HD, 1], bf16, tag="csb2", name="csb2")
                nc.vector.tensor_mul(csb[:], cps[:], bsb[:])
                r0 = h * c.HD
                while r0 < (h + 1) * c.HD:
                    kt = r0 // P
                    pr0 = r0 % P
                    take = min((h + 1) * c.HD - r0, P - pr0)
                    nc.sync.dma_start(
                        out=ctxL[pr0:pr0 + take, kt, b:b + 1],
                        in_=csb[r0 - h * c.HD:r0 - h * c.HD + take, :])
                    r0 += take

        # ---- slim o partial (row-oriented, streamed weights) + AR1 ----
        olrow = spool.tile([c.B, c.H], f32, tag="olrow", name="olrow",
                           bufs=1)
        ops2 = [psum.tile([c.B, cw], f32, tag="ps", name=f"os{ci}")
                for ci, (c0, cw) in enumerate(HCH)]
        for kt in range(c.DRT):
            for h0 in range(0, len(HCH), 3):
                his = HCH[h0:h0 + 3]
                wcols = sum(cw for _, cw in his)
                wt = wpool.tile([P, wcols], bf16, tag="wt", name="ow2")
                nc.sync.dma_start(
                    out=wt[:],
                    in_=wos[:, kt, his[0][0]:his[0][0] + wcols])
                for ci, (c0, cw) in enumerate(his):
                    nc.tensor.matmul(ops2[h0 + ci][:], ctxL[:, kt, :],
                                     wt[:, c0 - his[0][0]:c0 - his[0][0] + cw],
                                     start=(kt == 0), stop=(kt == c.DRT - 1))
        # fold residual x_last/NC (transpose own raw tiles inline)
        for ci, (c0, cw) in enumerate(HCH):
            xls = spool.tile([c.B, cw], bf16, tag="sg2", name="xls", bufs=1)
            for j in range(cw // P):
                tp = psum.tile([c.B, P], bf16, tag="ps", name=f"xlt{ci}{j}")
                nc.tensor.matmul(tp[:], xlraw[:, c0 // P + j, :],
                                 id128b_sb[:], is_transpose=True,
                                 start=True, stop=True)
                nc.vector.tensor_copy(xls[:, j * P:(j + 1) * P], tp[:])
            nc.vector.scalar_tensor_tensor(
                olrow[:, c0:c0 + cw], xls[:], invnc_col[:],
                ops2[ci][:], OP.mult, OP.add)
        olb = dram.tile([c.B, c.H], f32, tag="olb", name="olb")
        nc.sync.dma_start(out=olb[:], in_=olrow[:])
        xlg2 = dram.tile([c.B, c.H], f32, addr_space=SHARED,
                         tag="xlg2", name="xlg2")
        nc.gpsimd.collective_compute(
            "AllReduce", OP.add, replica_groups=RG,
            ins=[olb[:]], outs=[xlg2[:]])
        x2row = spool.tile([c.B, c.H], f32, tag="x2row", name="x2row",
                           bufs=1)
        nc.sync.dma_start(out=x2row[:], in_=xlg2[:])

        def row_rmsnorm(src_row, dst_row, tag):
            """src [B, H] bf16 -> dst [B, H] bf16 (rmsnorm, no weight)."""
            sq = spool.tile([c.B, c.H], bf16, tag="rsq", name=f"rsq{tag}",
                            bufs=1)
            nc.vector.tensor_mul(sq[:], src_row, src_row)


# revision 33
# speedup vs baseline: 1.3305x; 1.3305x over previous
"""Trainium2 Bass kernel for nn_Net_89361089561102 (2-layer dense transformer,
NF4-quantized weights, cls head). Tensor-parallel over 8 NeuronCores.

v2 design:
 - All heavy GEMMs run fp8(e4m3) DoubleRow (2 contraction k-tiles per
   instruction, ~1.7x effective PE throughput vs bf16). Weights are scaled
   x64 and activations x16 on the host / at conversion points; every GEMM
   epilogue folds the 1/1024 dequant plus the per-token rmsnorm scale.
 - rmsnorm is folded away: ln weights are pre-multiplied into the weight
   matrices host-side, and the per-token 1/rms scale is applied at GEMM
   outputs (pre-scaled cos/sin rope tables for q/k, per-token-column scale
   for v, broadcast-scale at the silu for gate/up). This removes the serial
   norm -> GEMM dependency everywhere: projections start on raw replicated
   fp8 activations immediately.
 - TP sharding as v1: qkv/gate_up output-dim sharded, o/down contract over
   AllGathered operands. ctx and intermediate AG payloads are fp8 (half the
   bytes); x-row AGs stay bf16 and carry the partial sum-of-squares row.
 - Slim last layer (only last token per batch flows through q/o/MLP),
   cls head fully replicated (no collective), exact own-row residual folds
   (no raw-x AllGather in the tail).
"""

import math
from contextlib import ExitStack
from dataclasses import dataclass

import numpy as np
import ml_dtypes

BF16 = ml_dtypes.bfloat16
FP8 = ml_dtypes.float8_e4m3
WS = 64.0     # weight fp8 scale
XS = 16.0     # activation fp8 scale
DQ = 1.0 / (WS * XS)
EPS = 1e-5
BLK = 64
NF4 = np.array([
    -1.0, -0.6961928009986877, -0.5250730514526367, -0.39491748809814453,
    -0.28444138169288635, -0.18477343022823334, -0.09105003625154495, 0.0,
    0.07958029955625534, 0.16093020141124725, 0.24611230194568634,
    0.33791524171829224, 0.44070982933044434, 0.5626170039176941,
    0.7229568362236023, 1.0], dtype=np.float32)


@dataclass(frozen=True)
class Cfg:
    H: int
    NH: int
    HD: int
    FF: int
    B: int
    S: int
    L: int
    NC: int
    CLS: int = 768
    NCLS: int = 2
    P: int = 128

    @property
    def T(self):
        return self.B * self.S

    @property
    def KT(self):
        return self.H // self.P

    @property
    def HPC(self):
        return self.NH // self.NC

    @property
    def DR(self):
        return self.HPC * self.HD

    @property
    def DRT(self):
        return self.DR // self.P

    @property
    def OR(self):
        return self.H // self.NC

    @property
    def OT(self):
        return self.OR // self.P

    @property
    def FPC(self):
        return self.FF // self.NC

    @property
    def FT(self):
        return self.FPC // self.P

    @property
    def KTF(self):
        return self.FF // self.P

    @property
    def SP(self):
        return self.S // self.P

    @property
    def TP_(self):
        return self.T // self.P

    @property
    def CT(self):
        return self.CLS // self.P

    def nchunks(self, M):
        n = (M + 511) // 512
        base = M // n
        assert base * n == M
        return [(i * base, base) for i in range(n)]

    def check(self):
        assert self.H % self.P == 0 and self.FF % self.P == 0
        assert self.S % self.P == 0 and self.S <= 512
        assert self.NH % self.NC == 0 and self.H % self.NC == 0
        assert self.FF % self.NC == 0
        assert self.DR % self.P == 0 or self.DR == self.P
        assert self.OR % self.P == 0 and self.FPC % self.P == 0
        assert self.HD <= self.P and self.HD % 2 == 0
        assert self.CLS % self.P == 0


FULL_CFG = Cfg(H=3072, NH=32, HD=96, FF=8192, B=2, S=512, L=2, NC=8)


def pairs(n):
    """[(k0, klen)] covering 0..n with klen 2 (DoubleRow) or 1 (trailing)."""
    out = []
    k = 0
    while k < n:
        kl = 2 if k + 2 <= n else 1
        out.append((k, kl))
        k += kl
    return out


# ----------------------------------------------------------------------------
# host-side prep
# ----------------------------------------------------------------------------

def dequant_np(packed, absmax, out_f, in_f):
    shifts = (np.arange(8, dtype=np.int32) * 4)
    codes = ((packed[:, None] >> shifts) & 0xF).reshape(-1)
    w = (NF4[codes].reshape(-1, BLK) * absmax[:, None].astype(np.float32))
    return w.reshape(out_f, in_f)


def _wpm(w_t, P, dt=FP8, scale=WS):
    """[K, M] fp32 -> [P, K//P, M] contiguous (partition-major), scaled."""
    K, M = w_t.shape
    w = np.clip(w_t * scale, -240.0, 240.0)
    return np.ascontiguousarray(
        w.reshape(K // P, P, M).transpose(1, 0, 2).astype(dt))


def host_prep(cfg: Cfg, inputs):
    c = cfg
    P = c.P
    x = inputs["embed"][inputs["input_ids"]]            # [B, S, H] fp32
    x0f = np.ascontiguousarray(x.reshape(c.T, c.H).T.astype(np.float32))
    x0q = np.ascontiguousarray(
        np.clip(x0f * XS, -240, 240).astype(FP8))       # [H, T] fp8 x16

    # rope tables (sign folded into sin)
    inv = 1.0 / (10000.0 ** (np.arange(0, c.HD, 2, dtype=np.float32) / c.HD))
    f = np.outer(np.arange(c.S, dtype=np.float32), inv)
    emb = np.concatenate([f, f], -1)
    sgn = np.concatenate([-np.ones(c.HD // 2, np.float32),
                          np.ones(c.HD // 2, np.float32)])
    cosT = np.ascontiguousarray(np.cos(emb).T.astype(np.float32))  # [HD, S]
    sinT = np.ascontiguousarray(
        (np.sin(emb).T * sgn[:, None]).astype(np.float32))
    cosT2 = np.ascontiguousarray(
        np.tile(cosT[:, c.S - 1:c.S], (1, c.B)).astype(np.float32))
    sinT2 = np.ascontiguousarray(
        np.tile(sinT[:, c.S - 1:c.S], (1, c.B)).astype(np.float32))

    am = (inputs["attention_mask"] != 0)                # [B, S]
    tk = np.arange(c.S)
    m1 = np.zeros((c.B, c.SP, P, c.S), np.float32)
    for b in range(c.B):
        for t in range(c.SP):
            rows = tk[t * P:(t + 1) * P]
            m1[b, t] = ((rows[:, None] <= tk[None, :]) & am[b, rows][:, None])
    m1 = m1.astype(BF16)
    am2 = np.zeros((c.B, P, c.SP), np.float32)
    for b in range(c.B):
        am2[b] = am[b].reshape(c.SP, P).T
    am2 = am2.astype(BF16)

    id128b = np.eye(P, dtype=BF16)
    id2b = np.eye(c.B, dtype=BF16)
    id2f = np.eye(c.B, dtype=np.float32)

    # cls head, fully replicated. final_ln folded into w1 columns.
    w1f = inputs["w1"].astype(np.float32) * inputs["final_ln_w"][None, :]
    w1t = _wpm(np.ascontiguousarray(w1f.T), P, dt=BF16, scale=1.0)
    b1row = np.ascontiguousarray(
        np.tile(inputs["b1"][None, :], (c.B, 1)).astype(np.float32))
    grow = np.ascontiguousarray(
        np.tile(inputs["ln_g"][None, :], (c.B, 1)).astype(np.float32))
    brow = np.ascontiguousarray(
        np.tile(inputs["ln_b"][None, :], (c.B, 1)).astype(np.float32))
    w2rows = np.ascontiguousarray(
        np.repeat(inputs["w2"].astype(np.float32)[None, :, :], c.B,
                  axis=0))                                # [B, NCLS, CLS]
    b2row = np.ascontiguousarray(
        np.tile(inputs["b2"][None, :], (c.B, 1)).astype(np.float32))

    shared = dict(x0q=x0q, cosT=cosT, sinT=sinT, cosT2=cosT2, sinT2=sinT2,
                  m1=m1, am2=am2, id128b=id128b, id2b=id2b, id2f=id2f,
                  w1t=w1t, b1row=b1row, grow=grow, brow=brow,
                  w2rows=w2rows, b2row=b2row)

    per_layer = []
    for l in range(c.L):
        wqkv = dequant_np(inputs["qkv_packed"][l], inputs["qkv_absmax"][l],
                          3 * c.H, c.H) * inputs["ln1_w"][l][None, :]
        wo = dequant_np(inputs["o_packed"][l], inputs["o_absmax"][l],
                        c.H, c.H)
        wgu = dequant_np(inputs["gu_packed"][l], inputs["gu_absmax"][l],
                         2 * c.FF, c.H) * inputs["ln2_w"][l][None, :]
        wd = dequant_np(inputs["down_packed"][l], inputs["down_absmax"][l],
                        c.H, c.FF)
        per_layer.append((wqkv, wo, wgu, wd))

    in_maps = []
    for core in range(c.NC):
        m = dict(shared)
        m["x0r"] = np.ascontiguousarray(
            x0f[core * c.OR:(core + 1) * c.OR, :])
        for l in range(c.L):
            wqkv, wo, wgu, wd = per_layer[l]
            d0 = core * c.DR
            m[f"wq{l}"] = _wpm(wqkv[d0:d0 + c.DR, :].T, P)
            m[f"wk{l}"] = _wpm(wqkv[c.H + d0:c.H + d0 + c.DR, :].T, P)
            m[f"wv{l}"] = _wpm(wqkv[2 * c.H + d0:2 * c.H + d0 + c.DR, :].T, P)
            o0 = core * c.OR
            g0 = core * c.FPC
            m[f"wg{l}"] = _wpm(wgu[g0:g0 + c.FPC, :].T, P)
            m[f"wu{l}"] = _wpm(wgu[c.FF + g0:c.FF + g0 + c.FPC, :].T, P)
            if l < c.L - 1:
                m[f"wo{l}"] = _wpm(wo[o0:o0 + c.OR, :].T, P)
                m[f"wd{l}"] = _wpm(wd[o0:o0 + c.OR, :].T, P)
            else:
                m["wos"] = _wpm(
                    np.ascontiguousarray(wo[:, d0:d0 + c.DR].T), P)
                m["wds"] = _wpm(
                    np.ascontiguousarray(wd[:, g0:g0 + c.FPC].T), P)
        in_maps.append(m)
    return in_maps


# ----------------------------------------------------------------------------
# device kernel
# ----------------------------------------------------------------------------

def build_nc(cfg: Cfg):
    import concourse.bass as bass
    import concourse.mybir as mybir
    import concourse.tile as tile
    from concourse import bacc

    c = cfg
    c.check()
    P = c.P
    f32 = mybir.dt.float32
    bf16 = mybir.dt.bfloat16
    fp8 = mybir.dt.float8e4
    AF = mybir.ActivationFunctionType
    OP = mybir.AluOpType
    DRMODE = mybir.MatmulPerfMode.DoubleRow

    nc = bacc.Bacc("TRN2", target_bir_lowering=False, debug=False,
                   enable_asserts=False, num_devices=c.NC)
    RG = [list(range(c.NC))]
    SHARED = "Shared" if c.NC > 4 else "Local"

    def din(name, shape, dt):
        return nc.dram_tensor(name, list(shape), dt, kind="ExternalInput").ap()

    x0q_d = din("x0q", [c.H, c.T], fp8)
    x0r = din("x0r", [c.OR, c.T], f32)
    cosT = din("cosT", [c.HD, c.S], f32)
    sinT = din("sinT", [c.HD, c.S], f32)
    cosT2 = din("cosT2", [c.HD, c.B], f32)
    sinT2 = din("sinT2", [c.HD, c.B], f32)
    m1 = din("m1", [c.B, c.SP, P, c.S], bf16)
    am2 = din("am2", [c.B, P, c.SP], bf16)
    id128b_d = din("id128b", [P, P], bf16)
    id2b_d = din("id2b", [c.B, c.B], bf16)
    id2f_d = din("id2f", [c.B, c.B], f32)
    w1t = din("w1t", [P, c.KT, c.CLS], bf16)
    b1row_d = din("b1row", [c.B, c.CLS], f32)
    grow_d = din("grow", [c.B, c.CLS], f32)
    brow_d = din("brow", [c.B, c.CLS], f32)
    w2rows_d = din("w2rows", [c.B, c.NCLS, c.CLS], f32)
    b2row_d = din("b2row", [c.B, c.NCLS], f32)
    wq = [din(f"wq{l}", [P, c.KT, c.DR], fp8) for l in range(c.L)]
    wk = [din(f"wk{l}", [P, c.KT, c.DR], fp8) for l in range(c.L)]
    wv = [din(f"wv{l}", [P, c.KT, c.DR], fp8) for l in range(c.L)]
    wo = [din(f"wo{l}", [P, c.KT, c.OR], fp8) for l in range(c.L - 1)]
    wos = din("wos", [P, c.DRT, c.H], fp8)
    wg = [din(f"wg{l}", [P, c.KT, c.FPC], fp8) for l in range(c.L)]
    wu = [din(f"wu{l}", [P, c.KT, c.FPC], fp8) for l in range(c.L)]
    wd = [din(f"wd{l}", [P, c.KTF, c.OR], fp8) for l in range(c.L - 1)]
    wds = din("wds", [P, c.FT, c.H], fp8)
    out_d = nc.dram_tensor("logits_out", [c.B, c.NCLS], f32,
                           kind="ExternalOutput").ap()

    isqrt_hd = 1.0 / math.sqrt(c.HD)
    HCH = c.nchunks(c.H)
    CCH = c.nchunks(c.CLS)
    KP = pairs(c.KT)       # contraction pairs over H
    FKP = pairs(c.KTF)     # contraction pairs over FF
    SLIMW = 16             # padded free width for slim fp8 stationary tiles

    with tile.TileContext(nc) as tc, ExitStack() as ctx:
        const = ctx.enter_context(tc.tile_pool(name="const", bufs=1))
        persist = ctx.enter_context(tc.tile_pool(name="persist", bufs=1))
        wpool = ctx.enter_context(tc.tile_pool(name="wpool", bufs=2))
        xpool = ctx.enter_context(tc.tile_pool(name="xpool", bufs=3))
        spool = ctx.enter_context(tc.tile_pool(name="spool", bufs=2))
        ppool = ctx.enter_context(tc.tile_pool(name="ppool", bufs=2))
        rpool = ctx.enter_context(tc.tile_pool(name="rpool", bufs=1))
        psum = ctx.enter_context(tc.tile_pool(name="psum", bufs=8,
                                              space="PSUM"))
        dram = ctx.enter_context(tc.tile_pool(name="dram", bufs=1,
                                              space="DRAM"))

        # ---- constants ----
        ones_cbf = const.tile([P, 1], bf16, tag="ones_cbf")
        nc.vector.memset(ones_cbf[:], 1.0)
        ones_r32 = const.tile([1, P], f32, tag="ones_r32")
        nc.vector.memset(ones_r32[:], 1.0)
        row16 = const.tile([1, P], f32, tag="row16")
        nc.vector.memset(row16[:], XS)
        rowdq = const.tile([1, P], f32, tag="rowdq")
        nc.vector.memset(rowdq[:], DQ)
        eps_col = const.tile([P, 1], f32, tag="eps_col")
        nc.vector.memset(eps_col[:], EPS)
        dq_col = const.tile([P, 1], f32, tag="dq_col")
        nc.vector.memset(dq_col[:], DQ)
        xs_col = const.tile([P, 1], f32, tag="xs_col")
        nc.vector.memset(xs_col[:], XS)
        invnc_col = const.tile([c.B, 1], f32, tag="invnc_col")
        nc.vector.memset(invnc_col[:], 1.0 / c.NC)
        cos_sb = const.tile([c.HD, c.S], f32, tag="cos_sb")
        nc.sync.dma_start(out=cos_sb[:], in_=cosT)
        sin_sb = const.tile([c.HD, c.S], f32, tag="sin_sb")
        nc.sync.dma_start(out=sin_sb[:], in_=sinT)
        cos2_sb = const.tile([c.HD, c.B], f32, tag="cos2_sb")
        nc.sync.dma_start(out=cos2_sb[:], in_=cosT2)
        sin2_sb = const.tile([c.HD, c.B], f32, tag="sin2_sb")
        nc.sync.dma_start(out=sin2_sb[:], in_=sinT2)
        am2_sb = const.tile([P, c.B, c.SP], bf16, tag="am2_sb")
        for b in range(c.B):
            nc.sync.dma_start(out=am2_sb[:, b, :], in_=am2[b])
        id128b_sb = const.tile([P, P], bf16, tag="id128b_sb")
        nc.sync.dma_start(out=id128b_sb[:], in_=id128b_d)
        id2b_sb = const.tile([c.B, c.B], bf16, tag="id2b_sb")
        nc.sync.dma_start(out=id2b_sb[:], in_=id2b_d)
        id2f_sb = const.tile([c.B, c.B], f32, tag="id2f_sb")
        nc.sync.dma_start(out=id2f_sb[:], in_=id2f_d)
        b1row_sb = const.tile([c.B, c.CLS], f32, tag="b1row_sb")
        nc.sync.dma_start(out=b1row_sb[:], in_=b1row_d)
        grow_sb = const.tile([c.B, c.CLS], f32, tag="grow_sb")
        nc.sync.dma_start(out=grow_sb[:], in_=grow_d)
        brow_sb = const.tile([c.B, c.CLS], f32, tag="brow_sb")
        nc.sync.dma_start(out=brow_sb[:], in_=brow_d)
        w2rows_sb = const.tile([c.B, c.NCLS, c.CLS], f32, tag="w2rows_sb")
        nc.sync.dma_start(out=w2rows_sb[:], in_=w2rows_d)
        b2row_sb = const.tile([c.B, c.NCLS], f32, tag="b2row_sb")
        nc.sync.dma_start(out=b2row_sb[:], in_=b2row_d)

        # ---- collective warm-up ----
        wu_sb = const.tile([16, 512], bf16, tag="wu_sb")
        nc.vector.memset(wu_sb[:], 0.0)
        wu_in = dram.tile([16, 512], bf16, tag="wu_in", name="wu_in")
        wu_out = dram.tile([16 * c.NC, 512], bf16, addr_space=SHARED,
                           tag="wu_out", name="wu_out")
        nc.sync.dma_start(out=wu_in[:], in_=wu_sb[:])
        nc.gpsimd.collective_compute(
            "AllGather", OP.bypass, replica_groups=RG,
            ins=[wu_in[:]], outs=[wu_out[:]])
        wu3_in = dram.tile([1, 512], bf16, tag="wu3_in", name="wu3_in")
        wu3_out = dram.tile([1, 512], bf16, addr_space=SHARED,
                            tag="wu3_out", name="wu3_out")
        nc.sync.dma_start(out=wu3_in[:], in_=wu_sb[0:1, :])
        nc.gpsimd.collective_compute(
            "AllReduce", OP.add, replica_groups=RG,
            ins=[wu3_in[:]], outs=[wu3_out[:]])

        # ---- persistent state ----
        xq = persist.tile([P, c.KT, c.T], fp8, tag="xq")       # x * XS
        xrows = persist.tile([P, c.OT, c.T], f32, tag="xrows")  # own raw rows
        for ot in range(c.OT):
            nc.sync.dma_start(out=xrows[:, ot, :],
                              in_=x0r[ot * P:(ot + 1) * P, :])
        GK = 6 if c.KT % 6 == 0 else 2
        for k0 in range(0, c.KT, GK):
            nc.sync.dma_start(
                out=xq[:, k0:k0 + GK, :],
                in_=x0q_d[k0 * P:(k0 + GK) * P, :].rearrange(
                    "(k p) t -> p k t", p=P))
        # scale tiles per batch: cosb/sinb (rope w/ s*DQ), bcs (s*DQ bcast),
        # s_col (per-token col scale), for the currently-relevant norm event
        cosb = persist.tile([c.HD, c.B, c.S], bf16, tag="cosb")
        sinb = persist.tile([c.HD, c.B, c.S], bf16, tag="sinb")
        bcs = persist.tile([P, c.B, c.S], bf16, tag="bcs")
        s_col = persist.tile([P, c.B, c.SP], f32, tag="s_col")

        # ---------- helpers ----------
        def ssq_rows(xsrc_fn, nkt, ncols, chunks, scale, tag):
            """sum over partitions+kt of xsrc^2 -> psum [1, cw] rows.
            xsrc_fn(kt) -> AP [P, ncols] (fp8 or bf16). scale multiplies
            inside Ln later; here raw accumulation."""
            ss = [psum.tile([1, cw], f32, tag="ps", name=f"ss{tag}{ci}")
                  for ci, (c0, cw) in enumerate(chunks)]
            for kt in range(nkt):
                src = xsrc_fn(kt)
                sq = xpool.tile([P, ncols], bf16, tag="sq", name=f"sq{tag}",
                                bufs=2)
                nc.vector.tensor_mul(sq[:], src, src)
                for ci, (c0, cw) in enumerate(chunks):
                    nc.tensor.matmul(ss[ci][:], ones_cbf[:], sq[:, c0:c0 + cw],
                                     start=(kt == 0), stop=(kt == nkt - 1))
            return ss

        def srow_from_ssq(ss_ps, cw, scale, tag):
            """psum ssq [1, cw] -> sbuf srow [1, cw] = rsqrt(ssq*scale+eps)."""
            lt = spool.tile([1, cw], f32, tag="lt", name=f"lt{tag}", bufs=2)
            nc.scalar.activation(lt[:], ss_ps[:], AF.Ln,
                                 bias=eps_col[0:1, :], scale=scale)
            rt = spool.tile([1, cw], f32, tag="rtrow", name=f"rt{tag}",
                            bufs=2)
            nc.scalar.activation(rt[:], lt[:], AF.Exp, scale=-0.5)
            return rt

        def bcast(row_ap, nrows, ncols, const_row, tag):
            """[1, ncols] -> psum [nrows, ncols] scaled by const_row value."""
            bb = psum.tile([nrows, ncols], f32, tag="ps", name=f"bb{tag}")
            nc.tensor.matmul(bb[:], const_row[:, 0:nrows], row_ap,
                             start=True, stop=True)
            return bb

        def scales_for(b, srow, tag, want_rope=True, want_bcs=True,
                       want_scol=True):
            """Fill cosb/sinb/bcs/s_col slices for batch b from srow [1,S]."""
            if want_rope:
                bbr = bcast(srow[:], c.HD, c.S, rowdq, f"r{tag}")
                nc.vector.tensor_mul(cosb[:, b, :], bbr[:], cos_sb[:])
                nc.vector.tensor_mul(sinb[:, b, :], bbr[:], sin_sb[:])
            if want_bcs:
                bbs = bcast(srow[:], P, c.S, rowdq, f"s{tag}")
                nc.scalar.copy(bcs[:, b, :], bbs[:])
            if want_scol:
                srdq = spool.tile([1, c.S], f32, tag="srdq",
                                  name=f"srdq{tag}", bufs=1)
                nc.vector.tensor_scalar_mul(srdq[:], srow[:], DQ)
                for t in range(c.SP):
                    tp = psum.tile([P, 1], f32, tag="ps", name=f"sc{tag}{t}")
                    nc.tensor.matmul(tp[:], srdq[:, t * P:(t + 1) * P],
                                     ones_r32[0:1, 0:1], is_transpose=True,
                                     start=True, stop=True)
                    nc.vector.tensor_copy(s_col[:, b, t:t + 1], tp[:])

        def fp8_kouter(kps, wsrc, wcols, groups, rhs_fn, name, rhs_load=None):
            """fp8 DoubleRow contraction over pairs kps, streaming weights.
            groups: list of (lhs_c0, lhs_cw, out_n, key). lhsT = weights.
            rhs_fn(k0, klen, key, rl) -> AP [P, klen, out_n]."""
            ps = [psum.tile([cw, n], f32, tag="ps", name=f"{name}{gi}")
                  for gi, (c0, cw, n, key) in enumerate(groups)]
            G = max(1, min(8, 2048 // wcols))  # pairs per DMA
            npair = len(kps)
            for p0 in range(0, npair, G):
                g_n = min(G, npair - p0)
                k0 = kps[p0][0]
                ktot = sum(kl for _, kl in kps[p0:p0 + g_n])
                wt = wpool.tile([P, 2 * G, wcols], fp8, tag="wt",
                                name=f"{name}w")
                nc.sync.dma_start(out=wt[:, 0:ktot, :],
                                  in_=wsrc(k0, ktot))
                for gi_p in range(g_n):
                    kp0, klen = kps[p0 + gi_p]
                    koff = kp0 - k0
                    first = (kp0 == 0)
                    last = (kp0 + klen == kps[-1][0] + kps[-1][1])
                    pm = DRMODE if klen == 2 else None
                    rl = rhs_load(kp0, klen) if rhs_load else None
                    for gi, (c0, cw, n, key) in enumerate(groups):
                        nc.tensor.matmul(
                            ps[gi][:],
                            wt[:, koff:koff + klen, c0:c0 + cw],
                            rhs_fn(kp0, klen, key, rl),
                            start=first, stop=last, perf_mode=pm)
            return ps

        def emit_rope(src_ps, dst, cos_ap, sin_ap, ncols, n=""):
            h2 = c.HD // 2
            qs = rpool.tile([c.HD, ncols], bf16, tag="qs", name=f"qs{n}",
                            bufs=2)
            nc.vector.tensor_copy(qs[:], src_ps[:])
            rot = rpool.tile([c.HD, ncols], bf16, tag="rot", name=f"rot{n}",
                             bufs=2)
            nc.sync.dma_start(out=rot[0:h2, :], in_=qs[h2:c.HD, :])
            nc.sync.dma_start(out=rot[h2:c.HD, :], in_=qs[0:h2, :])
            nc.vector.tensor_mul(qs[:], qs[:], cos_ap)
            nc.vector.tensor_mul(rot[:], rot[:], sin_ap)
            nc.vector.tensor_add(dst, qs[:], rot[:])

        def own_rows_ag(b, tag):
            """AllGather own raw x rows (bf16) + partial ssq row."""
            xnb = dram.tile([c.OR + 1, c.S], bf16, tag=f"xnb{tag}",
                            name=f"xnb{tag}")
            ss = psum.tile([1, c.S], f32, tag="ps", name=f"sso{tag}")
            for ot in range(c.OT):
                st = xpool.tile([P, c.S], bf16, tag="pst", name=f"sto{tag}",
                                bufs=3)
                xsl = xrows[:, ot, b * c.S:(b + 1) * c.S]
                nc.vector.tensor_copy(st[:], xsl)
                nc.sync.dma_start(out=xnb[ot * P:(ot + 1) * P, :], in_=st[:])
                sq = xpool.tile([P, c.S], bf16, tag="sq", name=f"sqo{tag}",
                                bufs=2)
                nc.vector.tensor_mul(sq[:], xsl, xsl)
                nc.tensor.matmul(ss[:], ones_cbf[:], sq[:],
                                 start=(ot == 0), stop=(ot == c.OT - 1))
            srow_p = spool.tile([1, c.S], bf16, tag="dr", name=f"srw{tag}",
                                bufs=1)
            nc.scalar.copy(srow_p[:], ss[:])
            nc.sync.dma_start(out=xnb[c.OR:c.OR + 1, :], in_=srow_p[:])
            xg = dram.tile([(c.OR + 1) * c.NC, c.S], bf16,
                           addr_space=SHARED, tag=f"xg{tag}",
                           name=f"xg{tag}")
            nc.gpsimd.collective_compute(
                "AllGather", OP.bypass, replica_groups=RG,
                ins=[xnb[:]], outs=[xg[:]])
            return xg

        def x_event_load(xg, b, tag, want_rope, want_bcs, want_scol):
            """Load gathered raw x rows -> xq fp8 (x*XS); derive srow and
            fill the scale tiles. Returns srow sbuf [1, S]."""
            sst = spool.tile([c.NC, c.S], bf16, tag="sst8", name=f"s8{tag}",
                             bufs=2)
            nc.sync.dma_start(
                out=sst[:],
                in_=xg[:].rearrange("(r q) s -> r q s",
                                    q=c.OR + 1)[:, c.OR, :])
            ssp = psum.tile([1, c.S], f32, tag="ps", name=f"ssp{tag}")
            nc.tensor.matmul(ssp[:], ones_cbf[0:c.NC, :], sst[:],
                             start=True, stop=True)
            srow = srow_from_ssq(ssp, c.S, 1.0 / c.H, tag)
            scales_for(b, srow, tag, want_rope, want_bcs, want_scol)
            for r in range(c.NC):
                stg = xpool.tile([P, c.OT, c.S], bf16, tag="stg",
                                 name=f"stg{tag}", bufs=2)
                nc.sync.dma_start(
                    out=stg[:],
                    in_=xg[r * (c.OR + 1):r * (c.OR + 1) + c.OR,
                           :].rearrange("(k p) s -> p k s", p=P))
                nc.vector.tensor_scalar_mul(
                    xq[:, r * c.OT:(r + 1) * c.OT, b * c.S:(b + 1) * c.S],
                    stg[:], XS)
            return srow

        # ================= initial scales from x0 =================
        full_chunks = [(b * c.S, c.S) for b in range(c.B)]
        ss0 = ssq_rows(lambda kt: xq[:, kt, :], c.KT, c.T, full_chunks,
                       None, "i")
        for b in range(c.B):
            srow0 = srow_from_ssq(ss0[b], c.S, 1.0 / (XS * XS * c.H), f"i{b}")
            scales_for(b, srow0, f"i{b}", want_bcs=False)

        # ================= full layers 0..L-2 =================
        for l in range(c.L - 1):
            # ---- q/k with rope, v ----
            q_rot = persist.tile([c.HD, c.HPC, c.T], bf16, tag="qrot",
                                 name=f"qrot{l}")
            k_rot = persist.tile([c.HD, c.HPC, c.T], bf16, tag="krot",
                                 name=f"krot{l}")
            v_sb = persist.tile([P, c.TP_, c.DR], bf16, tag="vsb",
                                name=f"vsb{l}")

            qg = [(h * c.HD, c.HD, c.S, b)
                  for h in range(c.HPC) for b in range(c.B)]
            qrhs = lambda k0, kl, b, rl: xq[:, k0:k0 + kl,
                                            b * c.S:(b + 1) * c.S]
            qps = fp8_kouter(KP, lambda k0, n: wq[l][:, k0:k0 + n, :],
                             c.DR, qg, qrhs, name="qp")
            for gi, (c0, cw, n, b) in enumerate(qg):
                h = c0 // c.HD
                emit_rope(qps[gi], q_rot[:, h, b * c.S:(b + 1) * c.S],
                          cosb[:, b, :], sinb[:, b, :], c.S)
            kps_ = fp8_kouter(KP, lambda k0, n: wk[l][:, k0:k0 + n, :],
                              c.DR, qg, qrhs, name="kp")
            for gi, (c0, cw, n, b) in enumerate(qg):
                h = c0 // c.HD
                emit_rope(kps_[gi], k_rot[:, h, b * c.S:(b + 1) * c.S],
                          cosb[:, b, :], sinb[:, b, :], c.S)
            # v: stationary = xq tiles, moving = weights
            vps = [psum.tile([P, c.DR], f32, tag="ps", name=f"vp{tt}")
                   for tt in range(c.TP_)]
            G = max(1, min(8, 2048 // c.DR))
            npair = len(KP)
            for p0 in range(0, npair, G):
                g_n = min(G, npair - p0)
                k0 = KP[p0][0]
                ktot = sum(kl for _, kl in KP[p0:p0 + g_n])
                wt = wpool.tile([P, 2 * G, c.DR], fp8, tag="wt", name="vw")
                nc.sync.dma_start(out=wt[:, 0:ktot, :],
                                  in_=wv[l][:, k0:k0 + ktot, :])
                for gi_p in range(g_n):
                    kp0, klen = KP[p0 + gi_p]
                    koff = kp0 - k0
                    pm = DRMODE if klen == 2 else None
                    for tt in range(c.TP_):
                        nc.tensor.matmul(
                            vps[tt][:],
                            xq[:, kp0:kp0 + klen, tt * P:(tt + 1) * P],
                            wt[:, koff:koff + klen, :],
                            start=(kp0 == 0),
                            stop=(kp0 + klen == c.KT), perf_mode=pm)
            for tt in range(c.TP_):
                b = tt // c.SP
                nc.vector.tensor_scalar(
                    v_sb[:, tt, :], vps[tt][:],
                    s_col[:, b, tt % c.SP:tt % c.SP + 1], None, OP.mult)

            # ---- attention (fp8 ctx out, x16) ----
            ctxgs = []
            for b in range(c.B):
                ctxb = dram.tile([c.DR, c.S], fp8, tag=f"ctxb{l}_{b}",
                                 name=f"ctxb{l}_{b}")
                mask_sb = ppool.tile([P, c.SP, c.S], bf16, tag="maskb",
                                     name=f"maskb{l}{b}", bufs=2)
                for t in range(c.SP):
                    nc.sync.dma_start(out=mask_sb[:, t, :], in_=m1[b, t])
                for h in range(c.HPC):
                    den = psum.tile([1, c.S], f32, tag="ps", name="den")
                    cps = psum.tile([c.HD, c.S], f32, tag="ps", name="cps")
                    for t in range(c.SP):
                        sps = psum.tile([P, c.S], f32, tag="ps", name="sps")
                        nc.tensor.matmul(
                            sps[:],
                            k_rot[:, h, b * c.S + t * P:
                                  b * c.S + (t + 1) * P],
                            q_rot[:, h, b * c.S:(b + 1) * c.S],
                            start=True, stop=True)
                        pt = ppool.tile([P, c.S], bf16, tag="pt", name="pt",
                                        bufs=3)
                        nc.scalar.activation(pt[:], sps[:], AF.Exp,
                                             scale=isqrt_hd)
                        nc.vector.tensor_mul(pt[:], pt[:], mask_sb[:, t, :])
                        nc.tensor.matmul(den[:], ones_cbf[:], pt[:],
                                         start=(t == 0),
                                         stop=(t == c.SP - 1))
                        nc.tensor.matmul(
                            cps[:],
                            v_sb[:, b * c.SP + t,
                                 h * c.HD:(h + 1) * c.HD],
                            pt[:],
                            start=(t == 0), stop=(t == c.SP - 1))
                    dr = spool.tile([1, c.S], f32, tag="dr", name="dr",
                                    bufs=1)
                    nc.vector.reciprocal(dr[:], den[:])
                    bb = bcast(dr[:], c.HD, c.S, row16, f"at{l}{b}{h}")
                    bsb = spool.tile([c.HD, c.S], bf16, tag="bsb",
                                     name="bsb", bufs=2)
                    nc.vector.tensor_copy(bsb[:], bb[:])
                    csb = spool.tile([c.HD, c.S], fp8, tag="csb",
                                     name="csb", bufs=1)
                    nc.vector.tensor_mul(csb[:], cps[:], bsb[:])
                    nc.sync.dma_start(
                        out=ctxb[h * c.HD:(h + 1) * c.HD, :], in_=csb[:])
                ctxg = dram.tile([c.H, c.S], fp8, addr_space=SHARED,
                                 tag=f"ctxg{l}_{b}", name=f"ctxg{l}_{b}")
                nc.gpsimd.collective_compute(
                    "AllGather", OP.bypass, replica_groups=RG,
                    ins=[ctxb[:]], outs=[ctxg[:]])
                ctxgs.append(ctxg)

            # ---- o (output-sharded over gathered fp8 ctx) + residual ----
            xgs = []
            for b in range(c.B):
                og = [(ot * P, P, c.S, b) for ot in range(c.OT)]

                def oload(k0, klen, _b=b):
                    t = xpool.tile([P, 2, c.S], fp8, tag="rhs", name="orhs",
                                   bufs=5)
                    nc.scalar.dma_start(
                        out=t[:, 0:klen, :],
                        in_=ctxgs[_b][k0 * P:(k0 + klen) * P, :].rearrange(
                            "(k p) s -> p k s", p=P))
                    return t
                ops_ = fp8_kouter(KP, lambda k0, n: wo[l][:, k0:k0 + n, :],
                                  c.OR, og,
                                  lambda k0, kl, key, rl: rl[:, 0:kl, :],
                                  name=f"op{b}", rhs_load=oload)
                for ot in range(c.OT):
                    xsl = xrows[:, ot, b * c.S:(b + 1) * c.S]
                    nc.vector.scalar_tensor_tensor(
                        xsl, ops_[ot][:], dq_col[:], xsl, OP.mult, OP.add)
                xgs.append(own_rows_ag(b, tag=f"o{l}_{b}"))

            # ---- gated MLP ----
            gact = persist.tile([P, c.FT, c.S], bf16, tag="gact",
                                name=f"gact{l}")
            intgs = []
            for b in range(c.B):
                x_event_load(xgs[b], b, tag=f"xo{l}{b}", want_rope=False,
                             want_bcs=True, want_scol=False)
                intb = dram.tile([c.FPC, c.S], fp8, tag=f"intb{l}_{b}",
                                 name=f"intb{l}_{b}")
                for phase, wsrc3 in (("g", wg[l]), ("u", wu[l])):
                    gg = [(ot * P, P, c.S, b) for ot in range(c.FT)]
                    grhs = (lambda k0, kl, _b, rl:
                            xq[:, k0:k0 + kl, _b * c.S:(_b + 1) * c.S])
                    gps = fp8_kouter(
                        KP, lambda k0, n, _w=wsrc3: _w[:, k0:k0 + n, :],
                        c.FPC, gg, grhs, name=f"{phase}{l}{b}")
                    for gi, (c0, cw, n, _b) in enumerate(gg):
                        ot = c0 // P
                        if phase == "g":
                            g2 = xpool.tile([P, c.S], bf16, tag="pst",
                                            name="g2", bufs=3)
                            nc.vector.tensor_mul(g2[:], gps[gi][:],
                                                 bcs[:, b, :])
                            sgt = xpool.tile([P, c.S], bf16, tag="sgt",
                                             name="sgt", bufs=3)
                            nc.scalar.activation(sgt[:], g2[:], AF.Sigmoid)
                            nc.vector.tensor_mul(gact[:, ot, :], g2[:],
                                                 sgt[:])
                        else:
                            t1 = xpool.tile([P, c.S], bf16, tag="pst",
                                            name="t1", bufs=3)
                            nc.vector.tensor_mul(t1[:], gps[gi][:],
                                                 gact[:, ot, :])
                            it = xpool.tile([P, c.S], fp8, tag="sgt",
                                            name="it", bufs=3)
                            nc.vector.scalar_tensor_tensor(
                                it[:], t1[:], xs_col[:], bcs[:, b, :],
                                OP.mult, OP.mult)
                            nc.sync.dma_start(
                                out=intb[ot * P:(ot + 1) * P, :], in_=it[:])
                intg = dram.tile([c.FF, c.S], fp8, addr_space=SHARED,
                                 tag=f"intg{l}_{b}", name=f"intg{l}_{b}")
                nc.gpsimd.collective_compute(
                    "AllGather", OP.bypass, replica_groups=RG,
                    ins=[intb[:]], outs=[intg[:]])
                intgs.append(intg)

            # ---- down (over gathered fp8 intermediate) + residual ----
            xgds = []
            for b in range(c.B):
                dg = [(ot * P, P, c.S, b) for ot in range(c.OT)]

                def dload(k0, klen, _b=b):
                    t = xpool.tile([P, 2, c.S], fp8, tag="rhs", name="drhs",
                                   bufs=5)
                    nc.scalar.dma_start(
                        out=t[:, 0:klen, :],
                        in_=intgs[_b][k0 * P:(k0 + klen) * P, :].rearrange(
                            "(k p) s -> p k s", p=P))
                    return t
                dps_ = fp8_kouter(FKP,
                                  lambda k0, n: wd[l][:, k0:k0 + n, :],
                                  c.OR, dg,
                                  lambda k0, kl, key, rl: rl[:, 0:kl, :],
                                  name=f"dp{b}", rhs_load=dload)
                for ot in range(c.OT):
                    xsl = xrows[:, ot, b * c.S:(b + 1) * c.S]
                    nc.vector.scalar_tensor_tensor(
                        xsl, dps_[ot][:], dq_col[:], xsl, OP.mult, OP.add)
                xgds.append(own_rows_ag(b, tag=f"d{l}_{b}"))

        # ================= slim last layer =================
        l = c.L - 1

        # transition load + L1 k/v per batch
        q_rot2 = persist.tile([c.HD, c.HPC, c.B], bf16, tag="qrot2",
                              name="qrot2")
        k_rot = persist.tile([c.HD, c.HPC, c.T], bf16, tag="krot",
                            name=f"krot{l}")
        v_sb = persist.tile([P, c.TP_, c.DR], bf16, tag="vsb",
                            name=f"vsb{l}")
        s2col = persist.tile([c.B, 1], f32, tag="s2col", name="s2col")
        for b in range(c.B):
            srow_d = x_event_load(xgds[b], b, tag=f"xd{b}", want_rope=True,
                                  want_bcs=False, want_scol=True)
            nc.sync.dma_start(out=s2col[b:b + 1, :],
                              in_=srow_d[:, c.S - 1:c.S])
            kg = [(h * c.HD, c.HD, c.S, b) for h in range(c.HPC)]
            krhs = (lambda k0, kl, _b, rl:
                    xq[:, k0:k0 + kl, _b * c.S:(_b + 1) * c.S])
            kps_ = fp8_kouter(KP, lambda k0, n: wk[l][:, k0:k0 + n, :],
                              c.DR, kg, krhs, name=f"kp2{b}")
            for gi, (c0, cw, n, _b) in enumerate(kg):
                h = c0 // c.HD
                emit_rope(kps_[gi], k_rot[:, h, b * c.S:(b + 1) * c.S],
                          cosb[:, b, :], sinb[:, b, :], c.S)
            vps = [psum.tile([P, c.DR], f32, tag="ps", name=f"vp2{b}{tt}")
                   for tt in range(c.SP)]
            G = max(1, min(8, 2048 // c.DR))
            npair = len(KP)
            for p0 in range(0, npair, G):
                g_n = min(G, npair - p0)
                k0 = KP[p0][0]
                ktot = sum(kl for _, kl in KP[p0:p0 + g_n])
                wt = wpool.tile([P, 2 * G, c.DR], fp8, tag="wt", name="vw2")
                nc.sync.dma_start(out=wt[:, 0:ktot, :],
                                  in_=wv[l][:, k0:k0 + ktot, :])
                for gi_p in range(g_n):
                    kp0, klen = KP[p0 + gi_p]
                    koff = kp0 - k0
                    pm = DRMODE if klen == 2 else None
                    for tt in range(c.SP):
                        tg = b * c.SP + tt
                        nc.tensor.matmul(
                            vps[tt][:],
                            xq[:, kp0:kp0 + klen, tg * P:(tg + 1) * P],
                            wt[:, koff:koff + klen, :],
                            start=(kp0 == 0), stop=(kp0 + klen == c.KT),
                            perf_mode=pm)
            for tt in range(c.SP):
                nc.vector.tensor_scalar(
                    v_sb[:, b * c.SP + tt, :], vps[tt][:],
                    s_col[:, b, tt:tt + 1], None, OP.mult)

        # slim q at last tokens: stationary = xq last cols (padded SLIMW)
        xql = persist.tile([P, c.KT, SLIMW], fp8, tag="xql", name="xql")
        nc.vector.memset(xql[:], 0.0)
        nc.vector.tensor_copy(
            xql[:, :, 0:c.B],
            xq[:].rearrange("p kt (b s) -> p kt b s", s=c.S)[:, :, :,
                                                             c.S - 1])
        qTp = psum.tile([c.B, c.DR], f32, tag="ps", name="qTp")
        G = max(1, min(8, 2048 // c.DR))
        npair = len(KP)
        for p0 in range(0, npair, G):
            g_n = min(G, npair - p0)
            k0 = KP[p0][0]
            ktot = sum(kl for _, kl in KP[p0:p0 + g_n])
            wt = wpool.tile([P, 2 * G, c.DR], fp8, tag="wt", name="qw2")
            nc.sync.dma_start(out=wt[:, 0:ktot, :],
                              in_=wq[l][:, k0:k0 + ktot, :])
            for gi_p in range(g_n):
                kp0, klen = KP[p0 + gi_p]
                koff = kp0 - k0
                pm = DRMODE if klen == 2 else None
                nc.tensor.matmul(qTp[:], xql[:, kp0:kp0 + klen, 0:c.B],
                                 wt[:, koff:koff + klen, :],
                                 start=(kp0 == 0), stop=(kp0 + klen == c.KT),
                                 perf_mode=pm)
        # scale rows by s_d(last token) * DQ then rope
        s2dq = spool.tile([c.B, 1], f32, tag="s2dq", name="s2dq")
        nc.vector.tensor_scalar_mul(s2dq[:], s2col[:], DQ)
        qTr = spool.tile([c.B, c.DR], f32, tag="qTr", name="qTr", bufs=1)
        nc.vector.tensor_scalar(qTr[:], qTp[:], s2dq[:], None, OP.mult)
        for h in range(c.HPC):
            qhp = psum.tile([c.HD, c.B], f32, tag="ps", name=f"qhp{h}")
            nc.tensor.matmul(qhp[:], qTr[:, h * c.HD:(h + 1) * c.HD],
                             id2f_sb[:], is_transpose=True,
                             start=True, stop=True)
            emit_rope(qhp, q_rot2[:, h, :], cos2_sb[:], sin2_sb[:], c.B,
                      n="2")

        # ---- slim attention -> ctxL fp8 [P, DRT, SLIMW] ----
        ctxL = persist.tile([P, c.DRT, SLIMW], fp8, tag="ctxL", name="ctxL")
        nc.vector.memset(ctxL[:], 0.0)
        for b in range(c.B):
            for h in range(c.HPC):
                sps = psum.tile([P, c.SP], f32, tag="ps", name="sps2")
                for t in range(c.SP):
                    nc.tensor.matmul(
                        sps[:, t:t + 1],
                        k_rot[:, h, b * c.S + t * P:b * c.S + (t + 1) * P],
                        q_rot2[:, h, b:b + 1],
                        start=True, stop=True)
                pt = ppool.tile([P, c.SP], bf16, tag="pt", name="pt2",
                                bufs=3)
                nc.scalar.activation(pt[:], sps[:], AF.Exp, scale=isqrt_hd)
                nc.vector.tensor_mul(pt[:], pt[:], am2_sb[:, b, :])
                dps = psum.tile([1, c.SP], f32, tag="ps", name="dps")
                nc.tensor.matmul(dps[:], ones_cbf[:], pt[:],
                                 start=True, stop=True)
                d1 = spool.tile([1, 1], f32, tag="d1", name="d1")
                nc.vector.tensor_reduce(d1[:], dps[:],
                                        mybir.AxisListType.X, OP.add)
                r1 = spool.tile([1, 1], f32, tag="r1", name="r1")
                nc.vector.reciprocal(r1[:], d1[:])
                nc.vector.tensor_scalar_mul(r1[:], r1[:], XS)
                cps = psum.tile([c.HD, 1], f32, tag="ps", name="cps2")
                for t in range(c.SP):
                    nc.tensor.matmul(
                        cps[:],
                        v_sb[:, b * c.SP + t, h * c.HD:(h + 1) * c.HD],
                        pt[:, t:t + 1],
                        start=(t == 0), stop=(t == c.SP - 1))
                bb = psum.tile([c.HD, 1], f32, tag="ps", name="bb2")
                nc.tensor.matmul(bb[:], ones_r32[:, 0:c.HD], r1[:],
                                 start=True, stop=True)
                bsb = spool.tile([c.HD, 1], f32, tag="bsb2", name="bsb2")
                nc.vector.tensor_copy(bsb[:], bb[:])
                csb = spool.tile([c.HD, 1], fp8, tag="csb2", name="csb2")
                nc.vector.tensor_mul(csb[:], cps[:], bsb[:])
                r0 = h * c.HD
                while r0 < (h + 1) * c.HD:
                    kt = r0 // P
                    pr0 = r0 % P
                    take = min((h + 1) * c.HD - r0, P - pr0)
                    nc.sync.dma_start(
                        out=ctxL[pr0:pr0 + take, kt, b:b + 1],
                        in_=csb[r0 - h * c.HD:r0 - h * c.HD + take, :])
                    r0 += take

        # ---- slim o partial (stationary ctxL, moving wos) + AR1 ----
        # raw x at last tokens: tiny AG of own rows' last columns
        xlb = dram.tile([c.OR, c.B], bf16, tag="xlb", name="xlb")
        for ot in range(c.OT):
            st = xpool.tile([P, c.B], bf16, tag="xlst", name="xlst", bufs=2)
            nc.vector.tensor_copy(
                st[:],
                xrows[:, ot, :].rearrange("p (b s) -> p b s",
                                          s=c.S)[:, :, c.S - 1])
            nc.sync.dma_start(out=xlb[ot * P:(ot + 1) * P, :], in_=st[:])
        xlg_raw = dram.tile([c.H, c.B], bf16, addr_space=SHARED,
                            tag="xlg_raw", name="xlg_raw")
        nc.gpsimd.collective_compute(
            "AllGather", OP.bypass, replica_groups=RG,
            ins=[xlb[:]], outs=[xlg_raw[:]])
        xlraw = persist.tile([P, c.KT, c.B], bf16, tag="xlraw",
                             name="xlraw")
        nc.sync.dma_start(
            out=xlraw[:],
            in_=xlg_raw[:].rearrange("(kt p) b -> p kt b", p=P))

        olb = dram.tile([c.B, c.H], f32, tag="olb", name="olb")
        DKP = pairs(c.DRT)
        ops2 = [psum.tile([c.B, cw], f32, tag="ps", name=f"os{ci}")
                for ci, (c0, cw) in enumerate(HCH)]
        for ci, (c0, cw) in enumerate(HCH):
            wt = wpool.tile([P, c.DRT, cw], fp8, tag="wos", name="wosw")
            nc.sync.dma_start(out=wt[:], in_=wos[:, :, c0:c0 + cw])
            for kp0, klen in DKP:
                pm = DRMODE if klen == 2 else None
                nc.tensor.matmul(
                    ops2[ci][:],
                    ctxL[:, kp0:kp0 + klen, 0:c.B],
                    wt[:, kp0:kp0 + klen, :],
                    start=(kp0 == 0), stop=(kp0 + klen == c.DRT), perf_mode=pm)
        # fold residual x_last/NC (transpose raw tiles inline)
        for ci, (c0, cw) in enumerate(HCH):
            xls = spool.tile([c.B, cw], bf16, tag="sg2", name="xls", bufs=1)
            for j in range(cw // P):
                tp = psum.tile([c.B, P], bf16, tag="ps", name=f"xlt{ci}{j}")
                nc.tensor.matmul(tp[:], xlraw[:, c0 // P + j, :],
                                 id128b_sb[:], is_transpose=True,
                                 start=True, stop=True)
                nc.vector.tensor_copy(xls[:, j * P:(j + 1) * P], tp[:])
            dsc = spool.tile([c.B, cw], f32, tag="sg2b", name=f"dsc{ci}",
                             bufs=1)
            nc.vector.tensor_scalar_mul(dsc[:], ops2[ci][:], DQ)
            och = spool.tile([c.B, cw], f32, tag="sg2c", name=f"och{ci}",
                             bufs=2)
            nc.vector.scalar_tensor_tensor(
                och[:], xls[:], invnc_col[:], dsc[:], OP.mult, OP.add)
            nc.sync.dma_start(out=olb[:, c0:c0 + cw], in_=och[:])

        xlg2 = dram.tile([c.B, c.H], f32, addr_space=SHARED,
                         tag="xlg2", name="xlg2")
        nc.gpsimd.collective_compute(
            "AllReduce", OP.add, replica_groups=RG,
            ins=[olb[:]], outs=[xlg2[:]])
        x2row = spool.tile([c.B, c.H], f32, tag="x2row", name="x2row",
                           bufs=1)
        nc.sync.dma_start(out=x2row[:], in_=xlg2[:])

        def row_rms_scale(src_row, tag, scale=1.0):
            """[B, H] -> [B, 1] rsqrt(mean^2+eps) * scale (chunked)."""
            ssr = spool.tile([c.B, 1], f32, tag="ssr", name=f"ssr{tag}")
            for ci, (c0, cw) in enumerate(HCH):
                sq = spool.tile([c.B, cw], f32, tag="rsq",
                                name=f"rsq{tag}{ci}", bufs=2)
                nc.vector.tensor_mul(sq[:], src_row[:, c0:c0 + cw],
                                     src_row[:, c0:c0 + cw])
                ssc = spool.tile([c.B, 1], f32, tag="ssc",
                                 name=f"ssc{tag}{ci}", bufs=2)
                nc.vector.tensor_reduce(ssc[:], sq[:], mybir.AxisListType.X,
                                        OP.add)
                if ci == 0:
                    nc.vector.tensor_copy(ssr[:], ssc[:])
                else:
                    nc.vector.tensor_add(ssr[:], ssr[:], ssc[:])
            lt = spool.tile([c.B, 1], f32, tag="lt2", name=f"lt2{tag}")
            nc.scalar.activation(lt[:], ssr[:], AF.Ln,
                                 bias=eps_col[0:c.B, :], scale=1.0 / c.H)
            rt = spool.tile([c.B, 1], f32, tag="rt2", name=f"rt2{tag}")
            nc.scalar.activation(rt[:], lt[:], AF.Exp, scale=-0.5)
            if scale != 1.0:
                nc.vector.tensor_scalar_mul(rt[:], rt[:], scale)
            return rt

        def row_to_tiles(src_row, nt, dst, tag, mulf=None):
            """[B, nt*P] row -> dst [P, nt, >=B] via PE transpose + DVE.
            src_row dtype selects the identity (f32 or bf16)."""
            srcdt = src_row.dtype
            ident = id2f_sb if srcdt == f32 else id2b_sb
            for j in range(nt):
                tp = psum.tile([P, c.B], srcdt, tag="ps", name=f"tt{tag}{j}")
                nc.tensor.matmul(tp[:], src_row[:, j * P:(j + 1) * P],
                                 ident[:], is_transpose=True,
                                 start=True, stop=True)
                if mulf is not None:
                    nc.vector.tensor_scalar_mul(dst[:, j, 0:c.B], tp[:],
                                                mulf)
                else:
                    nc.vector.tensor_copy(dst[:, j, 0:c.B], tp[:])

        # ---- slim gated MLP (stationary x2 cols fp8, moving weights) ----
        s2r = row_rms_scale(x2row[:], tag="l2", scale=DQ)
        xn2T = persist.tile([P, c.KT, SLIMW], fp8, tag="xql", name="xn2T")
        nc.vector.memset(xn2T[:], 0.0)
        row_to_tiles(x2row[:], c.KT, xn2T, tag="x2", mulf=XS)

        FCH = c.nchunks(c.FPC)
        garow = spool.tile([c.B, c.FPC], f32, tag="garow", name="garow",
                           bufs=1)
        for phase, wsrc3 in (("g", wg[l]), ("u", wu[l])):
            gps2 = [psum.tile([c.B, cw], f32, tag="ps", name=f"g2{phase}{ci}")
                    for ci, (c0, cw) in enumerate(FCH)]
            G = max(1, min(8, 2048 // c.FPC))
            npair = len(KP)
            for p0 in range(0, npair, G):
                g_n = min(G, npair - p0)
                k0 = KP[p0][0]
                ktot = sum(kl for _, kl in KP[p0:p0 + g_n])
                wt = wpool.tile([P, 2 * G, c.FPC], fp8, tag="wt",
                                name=f"g2w{phase}")
                nc.sync.dma_start(out=wt[:, 0:ktot, :],
                                  in_=wsrc3[:, k0:k0 + ktot, :])
                for gi_p in range(g_n):
                    kp0, klen = KP[p0 + gi_p]
                    koff = kp0 - k0
                    pm = DRMODE if klen == 2 else None
                    for ci, (c0, cw) in enumerate(FCH):
                        nc.tensor.matmul(gps2[ci][:],
                                         xn2T[:, kp0:kp0 + klen, 0:c.B],
                                         wt[:, koff:koff + klen, c0:c0 + cw],
                                         start=(kp0 == 0),
                                         stop=(kp0 + klen == c.KT),
                                         perf_mode=pm)
            for ci, (c0, cw) in enumerate(FCH):
                if phase == "g":
                    g2 = spool.tile([c.B, cw], f32, tag="sg2", name="g2s",
                                    bufs=1)
                    nc.vector.tensor_scalar(g2[:], gps2[ci][:], s2r[:],
                                            None, OP.mult)
                    sgt = spool.tile([c.B, cw], f32, tag="sg2b", name="sgt2",
                                     bufs=1)
                    nc.scalar.activation(sgt[:], g2[:], AF.Sigmoid)
                    nc.vector.tensor_mul(garow[:, c0:c0 + cw], g2[:], sgt[:])
                else:
                    t1 = spool.tile([c.B, cw], f32, tag="sg2", name="t1s",
                                    bufs=1)
                    nc.vector.tensor_scalar(t1[:], gps2[ci][:], s2r[:],
                                            None, OP.mult)
                    nc.vector.tensor_mul(garow[:, c0:c0 + cw],
                                         t1[:], garow[:, c0:c0 + cw])
        gactT = persist.tile([P, c.FT, SLIMW], fp8, tag="gactT",
                             name="gactT")
        nc.vector.memset(gactT[:], 0.0)
        row_to_tiles(garow[:], c.FT, gactT, tag="ga", mulf=XS)

        # ---- slim down partial + AR2 ----
        dlb = dram.tile([c.B, c.H], f32, tag="dlb", name="dlb")
        FTP = pairs(c.FT)
        dps2 = [psum.tile([c.B, cw], f32, tag="ps", name=f"ds{ci}")
                for ci, (c0, cw) in enumerate(HCH)]
        for ci, (c0, cw) in enumerate(HCH):
            wt = wpool.tile([P, c.FT, cw], fp8, tag="wt", name="wdsw")
            nc.sync.dma_start(out=wt[:], in_=wds[:, :, c0:c0 + cw])
            for kp0, klen in FTP:
                pm = DRMODE if klen == 2 else None
                nc.tensor.matmul(dps2[ci][:],
                                 gactT[:, kp0:kp0 + klen, 0:c.B],
                                 wt[:, kp0:kp0 + klen, :],
                                 start=(kp0 == 0), stop=(kp0 + klen == c.FT),
                                 perf_mode=pm)
            dch = spool.tile([c.B, cw], f32, tag="sg2c", name=f"dch{ci}",
                             bufs=2)
            nc.vector.tensor_scalar_mul(dch[:], dps2[ci][:], DQ)
            nc.sync.dma_start(out=dlb[:, c0:c0 + cw], in_=dch[:])
        xfing = dram.tile([c.B, c.H], f32, addr_space=SHARED,
                          tag="xfing", name="xfing")
        nc.gpsimd.collective_compute(
            "AllReduce", OP.add, replica_groups=RG,
            ins=[dlb[:]], outs=[xfing[:]])
        # in-place final residual: x2row += AR2 result
        for ci, (c0, cw) in enumerate(HCH):
            fch = spool.tile([c.B, cw], f32, tag="sg2c", name=f"fch{ci}",
                             bufs=2)
            nc.sync.dma_start(out=fch[:], in_=xfing[:, c0:c0 + cw])
            nc.vector.tensor_add(x2row[:, c0:c0 + cw],
                                 x2row[:, c0:c0 + cw], fch[:])

        # ================= final norm + replicated cls head =============
        sfr = row_rms_scale(x2row[:], tag="fin")
        xnf = persist.tile([P, c.KT, SLIMW], bf16, tag="xnf", name="xnf")
        nc.vector.memset(xnf[:], 0.0)
        row_to_tiles(x2row[:], c.KT, xnf, tag="xf")

        hps = [psum.tile([c.B, cw], f32, tag="ps", name=f"hp{ci}")
               for ci, (c0, cw) in enumerate(CCH)]
        GW1 = 3 if c.KT % 3 == 0 else c.KT
        for k0 in range(0, c.KT, GW1):
            wt1 = wpool.tile([P, GW1, c.CLS], bf16, tag="w1s", name="w1s")
            nc.sync.dma_start(out=wt1[:], in_=w1t[:, k0:k0 + GW1, :])
            for g in range(GW1):
                kt = k0 + g
                for ci, (c0, cw) in enumerate(CCH):
                    nc.tensor.matmul(hps[ci][:], xnf[:, kt, 0:c.B],
                                     wt1[:, g, c0:c0 + cw],
                                     start=(kt == 0), stop=(kt == c.KT - 1))
        hrow = spool.tile([c.B, c.CLS], f32, tag="hrow", name="hrow",
                          bufs=1)
        for ci, (c0, cw) in enumerate(CCH):
            t0_ = spool.tile([c.B, cw], f32, tag="sg2", name=f"hc{ci}",
                             bufs=1)
            nc.vector.scalar_tensor_tensor(
                t0_[:], hps[ci][:], sfr[:], b1row_sb[:, c0:c0 + cw],
                OP.mult, OP.add)
            nc.scalar.activation(hrow[:, c0:c0 + cw], t0_[:], AF.Relu)
        mu = spool.tile([c.B, 1], f32, tag="mu", name="mu")
        nc.vector.tensor_reduce(mu[:], hrow[:], mybir.AxisListType.X, OP.add)
        nc.vector.tensor_scalar_mul(mu[:], mu[:], 1.0 / c.CLS)
        hsq = spool.tile([c.B, c.CLS], f32, tag="hsq", name="hsq", bufs=1)
        nc.vector.tensor_mul(hsq[:], hrow[:], hrow[:])
        s2_ = spool.tile([c.B, 1], f32, tag="s2", name="s2")
        nc.vector.tensor_reduce(s2_[:], hsq[:], mybir.AxisListType.X, OP.add)
        nc.vector.tensor_scalar_mul(s2_[:], s2_[:], 1.0 / c.CLS)
        msq = spool.tile([c.B, 1], f32, tag="msq", name="msq")
        nc.vector.tensor_mul(msq[:], mu[:], mu[:])
        var = spool.tile([c.B, 1], f32, tag="var", name="var")
        nc.vector.tensor_sub(var[:], s2_[:], msq[:])
        lv = spool.tile([c.B, 1], f32, tag="lv", name="lv")
        nc.scalar.activation(lv[:], var[:], AF.Ln, bias=eps_col[0:c.B, :])
        rstd = spool.tile([c.B, 1], f32, tag="rstd", name="rstd")
        nc.scalar.activation(rstd[:], lv[:], AF.Exp, scale=-0.5)
        t1 = spool.tile([c.B, c.CLS], f32, tag="t1", name="t1f", bufs=1)
        nc.vector.tensor_scalar(t1[:], hrow[:], mu[:], rstd[:],
                                OP.subtract, OP.mult)
        hn = spool.tile([c.B, c.CLS], f32, tag="hsq", name="hn", bufs=1)
        nc.vector.tensor_mul(hn[:], t1[:], grow_sb[:])
        nc.vector.tensor_add(hn[:], hn[:], brow_sb[:])

        lg_sb = spool.tile([c.B, c.NCLS], f32, tag="lg_sb", name="lg_sb")
        for cc in range(c.NCLS):
            tmul = spool.tile([c.B, c.CLS], f32, tag="t1", name=f"tm{cc}",
                              bufs=1)
            nc.vector.tensor_mul(tmul[:], hn[:], w2rows_sb[:, cc, :])
            nc.vector.tensor_reduce(lg_sb[:, cc:cc + 1], tmul[:],
                                    mybir.AxisListType.X, OP.add)
        nc.vector.tensor_add(lg_sb[:], lg_sb[:], b2row_sb[:])
        nc.sync.dma_start(out=out_d, in_=lg_sb[:])

    nc.compile()
    return nc


# ----------------------------------------------------------------------------
# entry point
# ----------------------------------------------------------------------------

_CACHE = {}


def _get_nc(cfg):
    if cfg not in _CACHE:
        _CACHE[cfg] = build_nc(cfg)
    return _CACHE[cfg]


def run(cfg, inputs, trace=False, **kw):
    from concourse.bass_utils import run_bass_kernel_spmd
    in_maps = host_prep(cfg, inputs)
    nc = _get_nc(cfg)
    res = run_bass_kernel_spmd(nc, in_maps, core_ids=list(range(cfg.NC)),
                               trace=trace, **kw)
    out = np.asarray(res.results[0]["logits_out"])  # [B, NCLS]
    return np.ascontiguousarray(out.astype(np.float32)), res


def kernel(**inputs):
    inputs = {k: np.asarray(v) for k, v in inputs.items()}
    out, _ = run(FULL_CFG, inputs)
    return out
